# revision 1
# baseline (speedup 1.0000x reference)
"""MoE layer (routed top-2 of 8 experts) on 8 TRN2 NeuronCores.

Contract: kernel(**inputs) takes the FULL unsharded inputs and returns the
FULL [4, 4096, 512] float32 output. Sharding/compile/run happens inside.

Primary design (expert-parallel routed, MOE_DESIGN=routed, default):
  - Host computes the gating (128x128 @ 128x8 matmul, top-2, softmax) and
    per-expert token index lists.
  - Core e owns expert e: it gathers its expert's tokens directly from the
    full fp16 x in DRAM with a transposed dma_gather (tokens land as
    [din, token] tiles, exactly the matmul stationary layout), runs the
    512x512 expert matmul in fp16 with fp32 PSUM accumulation, adds the
    bias (DVE) and applies tanh (ACT, casting to fp16), and streams the
    dense gathered result yg = tanh(x[idx] @ We + be) back to DRAM.
  - Host applies the gate weights and scatter-adds each core's yg rows into
    the final fp32 output (each core's row set is duplicate-free, so this
    is a plain fancy-index add).

Fallback (MOE_DESIGN=dense): data-parallel dense-expert kernel in float32r
(~4x slower, ~2e-4 more accurate).
"""

import os

import numpy as np

# The axon NTFF profiling hooks (antenv.axon_hooks) are not shipped in this
# container; BASS_TRACE=1 in the environment would crash run_bass_kernel_spmd.
os.environ["BASS_NEVER_TRACE"] = "1"

import concourse.bass as bass
import concourse.bacc as bacc
import concourse.mybir as mybir
from concourse.tile import TileContext
from concourse.bass_utils import run_bass_kernel_spmd

F32 = mybir.dt.float32
F32R = mybir.dt.float32r
BF16 = mybir.dt.bfloat16
I16 = mybir.dt.int16
F16 = mybir.dt.float16

NB, NLOC, DIN, DOUT = 4, 4096, 512, 512
NTYPES, TEBD = 128, 128
NE, TOPK = 8, 2
NCORES = 8
T = NB * NLOC          # 16384 tokens
TC = T // NCORES       # 2048 tokens per core
MT = TC // 128         # 16 token m-tiles per core
KT = DIN // 128        # 4 k-tiles

_cache = {}

# set by run (module-level so test.py can read timing/trace results)
last_results = None


def _routing(type_embeddings, Wg, atom_types):
    """Host-side gating math (tiny): per-token dense expert weights [T, NE]."""
    logits = type_embeddings.astype(np.float32) @ Wg.astype(np.float32)  # [NTYPES, NE]
    order = np.argsort(-logits, axis=-1, kind="stable")                  # stable => ties to lower idx
    top2 = order[:, :TOPK]                                               # [NTYPES, 2]
    tv = np.take_along_axis(logits, top2, axis=-1)                       # [NTYPES, 2]
    ex = np.exp(tv - tv.max(axis=-1, keepdims=True))
    w = (ex / ex.sum(axis=-1, keepdims=True)).astype(np.float32)         # [NTYPES, 2]
    ptw_types = np.zeros((NTYPES, NE), np.float32)
    np.put_along_axis(ptw_types, top2, w, axis=-1)
    at = atom_types.reshape(-1)
    return ptw_types[at], top2[at], w[at]  # dense [T, NE], idx [T,2], w [T,2]


def _build_dense():
    """Dense data-parallel program: out[t,:] = sum_e pw[t,e]*tanh(x[t]@We[e]+be[e])."""
    nc = bacc.Bacc("TRN2", target_bir_lowering=False, debug=False)
    # xt and We are packed into ONE param/DMA so matmuls have a single
    # producer semaphore (the fp32r LW matmul only fits one sync wait).
    xw_d = nc.declare_dram_parameter("xw", [128, KT * TC + NE * KT * DOUT], F32R, isOutput=False)
    pwl_d = nc.declare_dram_parameter("pwl", [128, MT * NE], F32, isOutput=False)
    ber_d = nc.declare_dram_parameter("ber", [128, NE * DOUT], F32, isOutput=False)
    out_d = nc.declare_dram_parameter("out", [TC, DOUT], F32, isOutput=True)

    with TileContext(nc) as tc:
        with (
            tc.tile_pool(name="const", bufs=1) as cpool,
            tc.tile_pool(name="work", bufs=4) as wpool,
            tc.tile_pool(name="psum", bufs=1, space="PSUM") as ppool,
        ):
            xw_sb = cpool.tile([128, KT * TC + NE * KT * DOUT], F32R)
            nc.sync.dma_start(xw_sb[:], xw_d[:])
            XOFF = KT * TC
            pwl_sb = cpool.tile([128, MT * NE], F32)
            nc.sync.dma_start(pwl_sb[:], pwl_d[:])
            ber_sb = cpool.tile([128, NE * DOUT], F32)
            nc.sync.dma_start(ber_sb[:], ber_d[:])

            for m in range(MT):
                pss = []
                for e in range(NE):
                    pss.append(
                        ppool.tile([128, DOUT], F32, name=f"ps{e}", tag=f"ps{e}")
                    )
                for k in range(KT):
                    lhs = xw_sb[:, k * TC + m * 128 : k * TC + (m + 1) * 128]
                    for e in range(NE):
                        nc.tensor.matmul(
                            pss[e][:],
                            lhs,
                            xw_sb[:, XOFF + (e * KT + k) * DOUT : XOFF + (e * KT + k + 1) * DOUT],
                            start=(k == 0),
                            stop=(k == KT - 1),
                        )
                acc = wpool.tile([128, DOUT], F32, tag="acc")
                for e in range(NE):
                    t1 = wpool.tile([128, DOUT], F32, tag="t1")
                    nc.vector.tensor_add(t1[:], pss[e][:], ber_sb[:, bass.ts(e, DOUT)])
                    t2 = wpool.tile([128, DOUT], F32, tag="t2")
                    nc.scalar.activation(
                        t2[:], t1[:], mybir.ActivationFunctionType.Tanh
                    )
                    wsc = pwl_sb[:, m * NE + e : m * NE + e + 1]
                    if e == 0:
                        nc.vector.tensor_scalar_mul(acc[:], t2[:], wsc)
                    else:
                        nc.vector.scalar_tensor_tensor(
                            acc[:],
                            t2[:],
                            wsc,
                            acc[:],
                            op0=mybir.AluOpType.mult,
                            op1=mybir.AluOpType.add,
                        )
                nc.sync.dma_start(out_d[bass.ts(m, 128), :], acc[:])
    nc.compile()
    return nc


GCHUNK = 256  # tokens per dma_gather (balances ~1us SWDGE fixed cost vs pipelining)


def _build_routed(cap):
    """Expert-parallel routed program (one expert per core).

    Each core gathers its expert's tokens from the full fp16 x in DRAM via
    transposed dma_gather ([din, token] tiles), runs the expert matmul in
    fp16 (fp32 accumulate), applies bias+tanh, and writes the dense gathered
    output yg = tanh(x@We+be) [cap, 512] fp16.  The per-token gate weight and
    the scatter-add into the final output happen on host (row sets are unique
    per core, so it is a plain fancy-index add).
    """
    mte = cap // 128
    nc = bacc.Bacc("TRN2", target_bir_lowering=False, debug=False)
    xb_d = nc.declare_dram_parameter("xb", [T, DIN], F16, isOutput=False)
    we_d = nc.declare_dram_parameter("we", [128, KT * DOUT], F16, isOutput=False)
    ber_d = nc.declare_dram_parameter("ber", [128, DOUT], F32, isOutput=False)
    gidx_d = nc.declare_dram_parameter("gidx", [128, cap // 16], I16, isOutput=False)
    yg_d = nc.declare_dram_parameter("yg", [cap, DOUT], F16, isOutput=True)

    with TileContext(nc) as tc:
        with (
            tc.tile_pool(name="const", bufs=1) as cpool,
            tc.tile_pool(name="xg", bufs=4) as xgpool,
            tc.tile_pool(name="work", bufs=6) as wpool,
            tc.tile_pool(name="psum", bufs=1, space="PSUM") as ppool,
        ):
            # idx first: HWDGE DMAs drain FIFO per engine, and the gathers
            # (critical-path head) wait on the index table.
            idx_sb = cpool.tile([128, cap // 16], I16)
            nc.sync.dma_start(idx_sb[:], gidx_d[:])
            we_sb = cpool.tile([128, KT * DOUT], F16)
            nc.sync.dma_start(we_sb[:], we_d[:])
            ber_sb = cpool.tile([128, DOUT], F32)
            nc.sync.dma_start(ber_sb[:], ber_d[:])

            # Chunk pattern: two small 128-token warm-up gathers so the PE
            # starts ~2us sooner, then 256-token chunks (the measured HW
            # optimum), remainder last.
            chunks = [128, 128] if cap > 256 else [cap]
            rest = cap - sum(chunks)
            chunks += [GCHUNK] * (rest // GCHUNK)
            if rest % GCHUNK:
                chunks.append(rest % GCHUNK)
            g0 = 0
            m = 0
            for glen in chunks:
                xgm = xgpool.tile([128, KT, glen], F16, name="xgm", tag="xgm")
                nc.gpsimd.dma_gather(
                    out_ap=xgm[:],
                    in_ap=xb_d[:],
                    idxs_ap=idx_sb[:, g0 // 16 : (g0 + glen) // 16],
                    num_idxs=glen,
                    num_idxs_reg=glen,
                    elem_size=DIN,
                    transpose=True,
                )
                for off in range(0, glen, 128):
                    ps = ppool.tile(
                        [128, DOUT], F32, name=f"ps{m % 8}", tag=f"ps{m % 8}"
                    )
                    for k in range(KT):
                        nc.tensor.matmul(
                            ps[:],
                            xgm[:, k, off : off + 128],
                            we_sb[:, bass.ts(k, DOUT)],
                            start=(k == 0),
                            stop=(k == KT - 1),
                        )
                    t1 = wpool.tile([128, DOUT], F32, tag="t1")
                    nc.vector.tensor_add(t1[:], ps[:], ber_sb[:])
                    yg = wpool.tile([128, DOUT], F16, tag="yg")
                    nc.scalar.activation(
                        yg[:], t1[:], mybir.ActivationFunctionType.Tanh
                    )
                    nc.sync.dma_start(yg_d[bass.ts(m, 128), :], yg[:])
                    m += 1
                g0 += glen
    nc.compile()
    return nc


def _kernel_routed(x, type_embeddings, atom_types, Wg, We, be):
    global last_results
    x = np.asarray(x, np.float32)
    We = np.asarray(We, np.float32)
    be = np.asarray(be, np.float32)
    _, top2_t, w_t = _routing(
        np.asarray(type_embeddings, np.float32),
        np.asarray(Wg, np.float32),
        np.asarray(atom_types),
    )  # top2_t [T,2], w_t [T,2]

    x2 = x.reshape(T, DIN)
    xb = x2.astype(np.float16)

    # per-expert token lists (ascending token order)
    glist, gw = [], []
    for e in range(NE):
        sel1 = np.nonzero(top2_t[:, 0] == e)[0]
        sel2 = np.nonzero(top2_t[:, 1] == e)[0]
        toks = np.concatenate([sel1, sel2])
        ws = np.concatenate([w_t[sel1, 0], w_t[sel2, 1]])
        o = np.argsort(toks, kind="stable")
        glist.append(toks[o])
        gw.append(ws[o].astype(np.float32))
    counts = [len(g) for g in glist]
    cap = ((max(counts) + 127) // 128) * 128

    if ("routed", cap) not in _cache:
        _cache[("routed", cap)] = _build_routed(cap)
    nc = _cache[("routed", cap)]

    in_maps = []
    for e in range(NE):
        cnt = counts[e]
        gidx = np.zeros(cap, np.int16)
        gidx[:cnt] = glist[e]
        wvec = np.zeros(cap, np.float32)
        wvec[:cnt] = gw[e]
        # idx table: position i at [i % 16, i // 16], replicated to all 8
        # GPSIMD-core partition groups (HW reads per-core copies).
        idx16 = np.ascontiguousarray(
            np.tile(gidx.reshape(cap // 16, 16).T, (8, 1))
        ).astype(np.int16)
        we_c = np.ascontiguousarray(
            We[e].reshape(KT, 128, DOUT).transpose(1, 0, 2)
        ).reshape(128, KT * DOUT).astype(np.float16)
        ber = np.ascontiguousarray(
            np.broadcast_to(be[e].reshape(1, DOUT), (128, DOUT))
        )
        in_maps.append({"xb": xb, "we": we_c, "ber": ber, "gidx": idx16})

    res = run_bass_kernel_spmd(nc, in_maps, list(range(NCORES)))
    last_results = res

    out_full = np.zeros((T, DOUT), np.float32)
    for e in range(NE):
        cnt = counts[e]
        yg = np.asarray(res.results[e]["yg"][:cnt]).astype(np.float32)
        out_full[glist[e]] += gw[e][:cnt, None] * yg
    return out_full.reshape(NB, NLOC, DOUT)


def _build_routed2(tpc, nprim):
    """Load-balanced expert-parallel program.

    Every core computes `tpc` 128-token tiles: the first `nprim` use the
    core's resident primary-expert weights; the remaining `nov` tiles use
    per-tile weights (+bias) DMA'd from DRAM, letting overloaded experts
    spill whole tiles to under-loaded cores.  Same gather/epilogue as
    _build_routed.
    """
    nov = tpc - nprim
    cap = tpc * 128
    nc = bacc.Bacc("TRN2", target_bir_lowering=False, debug=False)
    xb_d = nc.declare_dram_parameter("xb", [T, DIN], F16, isOutput=False)
    we_d = nc.declare_dram_parameter("we", [128, KT * DOUT], F16, isOutput=False)
    ber_d = nc.declare_dram_parameter("ber", [128, DOUT], F32, isOutput=False)
    wem_d = nc.declare_dram_parameter(
        "wem", [128, nov * KT * DOUT], F16, isOutput=False
    )
    bem_d = nc.declare_dram_parameter("bem", [128, nov * DOUT], F16, isOutput=False)
    gidx_d = nc.declare_dram_parameter("gidx", [128, cap // 16], I16, isOutput=False)
    yg_d = nc.declare_dram_parameter("yg", [cap, DOUT], F16, isOutput=True)

    with TileContext(nc) as tc:
        with (
            tc.tile_pool(name="const", bufs=1) as cpool,
            tc.tile_pool(name="xg", bufs=4) as xgpool,
            tc.tile_pool(name="ow", bufs=3) as opool,
            tc.tile_pool(name="work", bufs=6) as wpool,
            tc.tile_pool(name="psum", bufs=1, space="PSUM") as ppool,
        ):
            idx_sb = cpool.tile([128, cap // 16], I16)
            nc.sync.dma_start(idx_sb[:], gidx_d[:])
            we_sb = cpool.tile([128, KT * DOUT], F16)
            nc.sync.dma_start(we_sb[:], we_d[:])
            ber_sb = cpool.tile([128, DOUT], F32)
            nc.sync.dma_start(ber_sb[:], ber_d[:])

            xgs = {}
            ow = {}
            for m in range(tpc):
                if m % (GCHUNK // 128) == 0:
                    g0 = m * 128
                    glen = min(GCHUNK, cap - g0)
                    xgm = xgpool.tile([128, KT, glen], F16, name="xgm", tag="xgm")
                    nc.gpsimd.dma_gather(
                        out_ap=xgm[:],
                        in_ap=xb_d[:],
                        idxs_ap=idx_sb[:, g0 // 16 : (g0 + glen) // 16],
                        num_idxs=glen,
                        num_idxs_reg=glen,
                        elem_size=DIN,
                        transpose=True,
                    )
                    xgs[m // (GCHUNK // 128)] = xgm
                xgm = xgs[m // (GCHUNK // 128)]
                off = (m % (GCHUNK // 128)) * 128
                if m < nprim:
                    wsrc, bsrc = we_sb, ber_sb[:]
                    woff = 0
                else:
                    # overflow tile: stream this slot's weights+bias from DRAM
                    # (emitted at use site so Tile schedules them just-in-time,
                    # prefetched `bufs` slots ahead, instead of up-front where
                    # they would starve the gathers of SDMA bandwidth)
                    j = m - nprim
                    wj = opool.tile([128, KT * DOUT], F16, name="wj", tag="wj")
                    nc.sync.dma_start(
                        wj[:], wem_d[:, j * KT * DOUT : (j + 1) * KT * DOUT]
                    )
                    bj = opool.tile([128, DOUT], F16, name="bj", tag="bj")
                    nc.sync.dma_start(bj[:], bem_d[:, j * DOUT : (j + 1) * DOUT])
                    wsrc, bsrc = wj, bj[:]
                    woff = 0
                ps = ppool.tile([128, DOUT], F32, name=f"ps{m % 8}", tag=f"ps{m % 8}")
                for k in range(KT):
                    nc.tensor.matmul(
                        ps[:],
                        xgm[:, k, off : off + 128],
                        wsrc[:, woff + k * DOUT : woff + (k + 1) * DOUT],
                        start=(k == 0),
                        stop=(k == KT - 1),
                    )
                t1 = wpool.tile([128, DOUT], F32, tag="t1")
                nc.vector.tensor_add(t1[:], ps[:], bsrc)
                yg = wpool.tile([128, DOUT], F16, tag="yg")
                nc.scalar.activation(yg[:], t1[:], mybir.ActivationFunctionType.Tanh)
                nc.sync.dma_start(yg_d[bass.ts(m, 128), :], yg[:])
    nc.compile()
    return nc


def _plan_balance(counts):
    """Pick (tpc, nprim) and assign each expert's 128-token tiles to cores.

    Returns (tpc, nprim, assign) where assign[c] is a list of length tpc of
    (expert, start, length) pieces ((c, 0, 0)-style dummies have length 0).
    Slot m < nprim must hold expert c (the core's resident expert); slots
    m >= nprim may hold any expert (weights come via the wem input).
    """
    ne = len(counts)
    ceils = [(c + 127) // 128 for c in counts]
    total = sum(ceils)
    tpc0 = max((total + ne - 1) // ne, 1)
    for tpc in range(tpc0, tpc0 + 64):
        nprim = None
        for cand in range(tpc, -1, -1):
            spill = sum(max(ce - cand, 0) for ce in ceils)
            if spill <= ne * (tpc - cand):
                nprim = cand
                break
        if nprim is not None:
            break
    assert nprim is not None
    # primary slots: expert c's first min(ceil_c, nprim) tiles on core c
    assign = []
    spill_tiles = []
    for e in range(ne):
        nown = min(ceils[e], nprim)
        tiles = [(e, t * 128, min(128, counts[e] - t * 128)) for t in range(ceils[e])]
        own = tiles[:nown] + [(e, 0, 0)] * (nprim - nown)
        assign.append(own)
        spill_tiles.extend(tiles[nown:])
    # overflow slots round-robin
    nov = tpc - nprim
    for c in range(ne):
        take, spill_tiles = spill_tiles[:nov], spill_tiles[nov:]
        take = take + [(c, 0, 0)] * (nov - take.__len__())
        assign[c] = assign[c] + take
    assert not spill_tiles
    return tpc, nprim, assign


def _kernel_routed2(x, type_embeddings, atom_types, Wg, We, be):
    global last_results
    x = np.asarray(x, np.float32)
    We = np.asarray(We, np.float32)
    be = np.asarray(be, np.float32)
    _, top2_t, w_t = _routing(
        np.asarray(type_embeddings, np.float32),
        np.asarray(Wg, np.float32),
        np.asarray(atom_types),
    )
    xb = x.reshape(T, DIN).astype(np.float16)

    glist, gw = [], []
    for e in range(NE):
        sel1 = np.nonzero(top2_t[:, 0] == e)[0]
        sel2 = np.nonzero(top2_t[:, 1] == e)[0]
        toks = np.concatenate([sel1, sel2])
        ws = np.concatenate([w_t[sel1, 0], w_t[sel2, 1]])
        o = np.argsort(toks, kind="stable")
        glist.append(toks[o])
        gw.append(ws[o].astype(np.float32))
    counts = [len(g) for g in glist]

    tpc, nprim, assign = _plan_balance(counts)
    nov = tpc - nprim
    cap = tpc * 128
    if ("routed2", tpc, nprim) not in _cache:
        _cache[("routed2", tpc, nprim)] = _build_routed2(tpc, nprim)
    nc = _cache[("routed2", tpc, nprim)]

    we_h = [
        np.ascontiguousarray(We[e].reshape(KT, 128, DOUT).transpose(1, 0, 2))
        .reshape(128, KT * DOUT)
        .astype(np.float16)
        for e in range(NE)
    ]
    ber_h = [
        np.ascontiguousarray(np.broadcast_to(be[e].reshape(1, DOUT), (128, DOUT)))
        for e in range(NE)
    ]
    in_maps = []
    for c in range(NCORES):
        gidx = np.zeros(cap, np.int16)
        for m, (e, s, L) in enumerate(assign[c]):
            if L:
                gidx[m * 128 : m * 128 + L] = glist[e][s : s + L]
        idx16 = np.ascontiguousarray(
            np.tile(gidx.reshape(cap // 16, 16).T, (8, 1))
        ).astype(np.int16)
        wem = np.concatenate(
            [we_h[e] for (e, s, L) in assign[c][nprim:]], axis=1
        ) if nov else np.zeros((128, 0), np.float16)
        bem = np.concatenate(
            [ber_h[e].astype(np.float16) for (e, s, L) in assign[c][nprim:]], axis=1
        ) if nov else np.zeros((128, 0), np.float16)
        in_maps.append(
            {
                "xb": xb,
                "we": we_h[c],
                "ber": ber_h[c],
                "wem": np.ascontiguousarray(wem),
                "bem": np.ascontiguousarray(bem),
                "gidx": idx16,
            }
        )

    res = run_bass_kernel_spmd(nc, in_maps, list(range(NCORES)))
    last_results = res

    out_full = np.zeros((T, DOUT), np.float32)
    # accumulate per expert (each expert's tiles partition its token list,
    # so indices are unique within one fancy-index add)
    for e in range(NE):
        ids, rows, ws = [], [], []
        for c in range(NCORES):
            yg = None
            for m, (te, s, L) in enumerate(assign[c]):
                if te == e and L:
                    if yg is None:
                        yg = np.asarray(res.results[c]["yg"])
                    ids.append(glist[e][s : s + L])
                    rows.append(yg[m * 128 : m * 128 + L])
                    ws.append(gw[e][s : s + L])
        if ids:
            ids = np.concatenate(ids)
            rows = np.concatenate(rows).astype(np.float32)
            ws = np.concatenate(ws)
            out_full[ids] += ws[:, None] * rows
    return out_full.reshape(NB, NLOC, DOUT)


def kernel(x, type_embeddings, atom_types, Wg, We, be):
    global last_results
    design = os.environ.get("MOE_DESIGN", "routed")
    if design == "routed2":
        return _kernel_routed2(x, type_embeddings, atom_types, Wg, We, be)
    if design == "routed":
        return _kernel_routed(x, type_embeddings, atom_types, Wg, We, be)
    x = np.asarray(x, np.float32)
    We = np.asarray(We, np.float32)
    be = np.asarray(be, np.float32)
    ptw, _, _ = _routing(
        np.asarray(type_embeddings, np.float32),
        np.asarray(Wg, np.float32),
        np.asarray(atom_types),
    )

    x2 = x.reshape(T, DIN)
    ber = np.ascontiguousarray(
        np.broadcast_to(be.reshape(1, NE * DOUT), (128, NE * DOUT))
    )
    # [128, NE*KT*DOUT]: we_h[p, (e*KT+k)*DOUT + d] = We[e, k*128+p, d]
    we_h = np.ascontiguousarray(
        We.reshape(NE, KT, 128, DOUT).transpose(2, 0, 1, 3)
    ).reshape(128, NE * KT * DOUT)
    in_maps = []
    for c in range(NCORES):
        x2c = x2[c * TC : (c + 1) * TC]
        # [128, KT*TC]: xt[p, k*TC + n] = x2c[n, k*128+p]
        xt = np.ascontiguousarray(
            x2c.reshape(TC, KT, 128).transpose(2, 1, 0)
        ).reshape(128, KT * TC)
        xw = np.concatenate([xt, we_h], axis=1)
        pwl = np.ascontiguousarray(
            ptw[c * TC : (c + 1) * TC].reshape(MT, 128, NE).transpose(1, 0, 2)
        ).reshape(128, MT * NE)
        in_maps.append({"xw": xw, "pwl": pwl, "ber": ber})

    if "dense" not in _cache:
        _cache["dense"] = _build_dense()
    nc = _cache["dense"]

    res = run_bass_kernel_spmd(nc, in_maps, list(range(NCORES)))
    last_results = res
    out = np.concatenate([res.results[c]["out"] for c in range(NCORES)], axis=0)
    return out.reshape(NB, NLOC, DOUT).astype(np.float32)



# revision 25
# speedup vs baseline: 1.2980x; 1.2980x over previous
"""MoE layer (routed top-2 of 8 experts) on 8 TRN2 NeuronCores.

Contract: kernel(**inputs) takes the FULL unsharded inputs and returns the
FULL [4, 4096, 512] float32 output. Sharding/compile/run happens inside.

Primary design (expert-parallel routed, MOE_DESIGN=routed, default):
  - Host computes the gating (128x128 @ 128x8 matmul, top-2, softmax) and
    per-expert token index lists.
  - Core e owns expert e: it gathers its expert's tokens directly from the
    full fp16 x in DRAM with a transposed dma_gather (tokens land as
    [din, token] tiles, exactly the matmul stationary layout), runs the
    512x512 expert matmul in fp16 with fp32 PSUM accumulation, adds the
    bias (DVE) and applies tanh (ACT, casting to fp16), and streams the
    dense gathered result yg = tanh(x[idx] @ We + be) back to DRAM.
  - Host applies the gate weights and scatter-adds each core's yg rows into
    the final fp32 output (each core's row set is duplicate-free, so this
    is a plain fancy-index add).

Fallback (MOE_DESIGN=dense): data-parallel dense-expert kernel in float32r
(~4x slower, ~2e-4 more accurate).
"""

import os

import numpy as np

# The axon NTFF profiling hooks (antenv.axon_hooks) are not shipped in this
# container; BASS_TRACE=1 in the environment would crash run_bass_kernel_spmd.
os.environ["BASS_NEVER_TRACE"] = "1"

import concourse.bass as bass
import concourse.bacc as bacc
import concourse.mybir as mybir
from concourse.tile import TileContext
from concourse.bass_utils import run_bass_kernel_spmd

F32 = mybir.dt.float32
F32R = mybir.dt.float32r
BF16 = mybir.dt.bfloat16
I16 = mybir.dt.int16
F16 = mybir.dt.float16

NB, NLOC, DIN, DOUT = 4, 4096, 512, 512
NTYPES, TEBD = 128, 128
NE, TOPK = 8, 2
NCORES = 8
T = NB * NLOC          # 16384 tokens
TC = T // NCORES       # 2048 tokens per core
MT = TC // 128         # 16 token m-tiles per core
KT = DIN // 128        # 4 k-tiles

_cache = {}

# set by run (module-level so test.py can read timing/trace results)
last_results = None


def _routing(type_embeddings, Wg, atom_types):
    """Host-side gating math (tiny): per-token dense expert weights [T, NE]."""
    logits = type_embeddings.astype(np.float32) @ Wg.astype(np.float32)  # [NTYPES, NE]
    order = np.argsort(-logits, axis=-1, kind="stable")                  # stable => ties to lower idx
    top2 = order[:, :TOPK]                                               # [NTYPES, 2]
    tv = np.take_along_axis(logits, top2, axis=-1)                       # [NTYPES, 2]
    ex = np.exp(tv - tv.max(axis=-1, keepdims=True))
    w = (ex / ex.sum(axis=-1, keepdims=True)).astype(np.float32)         # [NTYPES, 2]
    ptw_types = np.zeros((NTYPES, NE), np.float32)
    np.put_along_axis(ptw_types, top2, w, axis=-1)
    at = atom_types.reshape(-1)
    return ptw_types[at], top2[at], w[at]  # dense [T, NE], idx [T,2], w [T,2]


def _build_dense():
    """Dense data-parallel program: out[t,:] = sum_e pw[t,e]*tanh(x[t]@We[e]+be[e])."""
    nc = bacc.Bacc("TRN2", target_bir_lowering=False, debug=False)
    # xt and We are packed into ONE param/DMA so matmuls have a single
    # producer semaphore (the fp32r LW matmul only fits one sync wait).
    xw_d = nc.declare_dram_parameter("xw", [128, KT * TC + NE * KT * DOUT], F32R, isOutput=False)
    pwl_d = nc.declare_dram_parameter("pwl", [128, MT * NE], F32, isOutput=False)
    ber_d = nc.declare_dram_parameter("ber", [128, NE * DOUT], F32, isOutput=False)
    out_d = nc.declare_dram_parameter("out", [TC, DOUT], F32, isOutput=True)

    with TileContext(nc) as tc:
        with (
            tc.tile_pool(name="const", bufs=1) as cpool,
            tc.tile_pool(name="work", bufs=4) as wpool,
            tc.tile_pool(name="psum", bufs=1, space="PSUM") as ppool,
        ):
            xw_sb = cpool.tile([128, KT * TC + NE * KT * DOUT], F32R)
            nc.sync.dma_start(xw_sb[:], xw_d[:])
            XOFF = KT * TC
            pwl_sb = cpool.tile([128, MT * NE], F32)
            nc.sync.dma_start(pwl_sb[:], pwl_d[:])
            ber_sb = cpool.tile([128, NE * DOUT], F32)
            nc.sync.dma_start(ber_sb[:], ber_d[:])

            for m in range(MT):
                pss = []
                for e in range(NE):
                    pss.append(
                        ppool.tile([128, DOUT], F32, name=f"ps{e}", tag=f"ps{e}")
                    )
                for k in range(KT):
                    lhs = xw_sb[:, k * TC + m * 128 : k * TC + (m + 1) * 128]
                    for e in range(NE):
                        nc.tensor.matmul(
                            pss[e][:],
                            lhs,
                            xw_sb[:, XOFF + (e * KT + k) * DOUT : XOFF + (e * KT + k + 1) * DOUT],
                            start=(k == 0),
                            stop=(k == KT - 1),
                        )
                acc = wpool.tile([128, DOUT], F32, tag="acc")
                for e in range(NE):
                    t1 = wpool.tile([128, DOUT], F32, tag="t1")
                    nc.vector.tensor_add(t1[:], pss[e][:], ber_sb[:, bass.ts(e, DOUT)])
                    t2 = wpool.tile([128, DOUT], F32, tag="t2")
                    nc.scalar.activation(
                        t2[:], t1[:], mybir.ActivationFunctionType.Tanh
                    )
                    wsc = pwl_sb[:, m * NE + e : m * NE + e + 1]
                    if e == 0:
                        nc.vector.tensor_scalar_mul(acc[:], t2[:], wsc)
                    else:
                        nc.vector.scalar_tensor_tensor(
                            acc[:],
                            t2[:],
                            wsc,
                            acc[:],
                            op0=mybir.AluOpType.mult,
                            op1=mybir.AluOpType.add,
                        )
                nc.sync.dma_start(out_d[bass.ts(m, 128), :], acc[:])
    nc.compile()
    return nc


GCHUNK = 256  # tokens per dma_gather (balances ~1us SWDGE fixed cost vs pipelining)


def _build_routed(cap):
    """Expert-parallel routed program (one expert per core).

    Each core gathers its expert's tokens from the full fp16 x in DRAM via
    transposed dma_gather ([din, token] tiles), runs the expert matmul in
    fp16 (fp32 accumulate), applies bias+tanh, and writes the dense gathered
    output yg = tanh(x@We+be) [cap, 512] fp16.  The per-token gate weight and
    the scatter-add into the final output happen on host (row sets are unique
    per core, so it is a plain fancy-index add).
    """
    mte = cap // 128
    nc = bacc.Bacc("TRN2", target_bir_lowering=False, debug=False)
    xb_d = nc.declare_dram_parameter("xb", [T, DIN], F16, isOutput=False)
    we_d = nc.declare_dram_parameter("we", [128, KT * DOUT], F16, isOutput=False)
    ber_d = nc.declare_dram_parameter("ber", [128, DOUT], F32, isOutput=False)
    gidx_d = nc.declare_dram_parameter("gidx", [128, cap // 16], I16, isOutput=False)
    yg_d = nc.declare_dram_parameter("yg", [cap, DOUT], F16, isOutput=True)

    with TileContext(nc) as tc:
        with (
            tc.tile_pool(name="const", bufs=1) as cpool,
            tc.tile_pool(name="xg", bufs=4) as xgpool,
            tc.tile_pool(name="work", bufs=6) as wpool,
            tc.tile_pool(name="psum", bufs=1, space="PSUM") as ppool,
        ):
            # idx first: HWDGE DMAs drain FIFO per engine, and the gathers
            # (critical-path head) wait on the index table.
            idx_sb = cpool.tile([128, cap // 16], I16)
            nc.sync.dma_start(idx_sb[:], gidx_d[:])
            we_sb = cpool.tile([128, KT * DOUT], F16)
            nc.sync.dma_start(we_sb[:], we_d[:])
            ber_sb = cpool.tile([128, DOUT], F32)
            nc.sync.dma_start(ber_sb[:], ber_d[:])

            # Chunk pattern: two small 128-token warm-up gathers so the PE
            # starts ~2us sooner, then 256-token chunks (the measured HW
            # optimum), remainder last.
            chunks = [128, 128] if cap > 256 else [cap]
            rest = cap - sum(chunks)
            chunks += [GCHUNK] * (rest // GCHUNK)
            if rest % GCHUNK:
                chunks.append(rest % GCHUNK)
            g0 = 0
            m = 0
            for glen in chunks:
                xgm = xgpool.tile([128, KT, glen], F16, name="xgm", tag="xgm")
                nc.gpsimd.dma_gather(
                    out_ap=xgm[:],
                    in_ap=xb_d[:],
                    idxs_ap=idx_sb[:, g0 // 16 : (g0 + glen) // 16],
                    num_idxs=glen,
                    num_idxs_reg=glen,
                    elem_size=DIN,
                    transpose=True,
                )
                for off in range(0, glen, 128):
                    ps = ppool.tile(
                        [128, DOUT], F32, name=f"ps{m % 8}", tag=f"ps{m % 8}"
                    )
                    for k in range(KT):
                        nc.tensor.matmul(
                            ps[:],
                            xgm[:, k, off : off + 128],
                            we_sb[:, bass.ts(k, DOUT)],
                            start=(k == 0),
                            stop=(k == KT - 1),
                        )
                    t1 = wpool.tile([128, DOUT], F32, tag="t1")
                    nc.vector.tensor_add(t1[:], ps[:], ber_sb[:])
                    yg = wpool.tile([128, DOUT], F16, tag="yg")
                    nc.scalar.activation(
                        yg[:], t1[:], mybir.ActivationFunctionType.Tanh
                    )
                    nc.sync.dma_start(yg_d[bass.ts(m, 128), :], yg[:])
                    m += 1
                g0 += glen
    nc.compile()
    return nc


def _kernel_routed(x, type_embeddings, atom_types, Wg, We, be):
    global last_results
    x = np.asarray(x, np.float32)
    We = np.asarray(We, np.float32)
    be = np.asarray(be, np.float32)
    _, top2_t, w_t = _routing(
        np.asarray(type_embeddings, np.float32),
        np.asarray(Wg, np.float32),
        np.asarray(atom_types),
    )  # top2_t [T,2], w_t [T,2]

    x2 = x.reshape(T, DIN)
    xb = x2.astype(np.float16)

    # per-expert token lists (ascending token order)
    glist, gw = [], []
    for e in range(NE):
        sel1 = np.nonzero(top2_t[:, 0] == e)[0]
        sel2 = np.nonzero(top2_t[:, 1] == e)[0]
        toks = np.concatenate([sel1, sel2])
        ws = np.concatenate([w_t[sel1, 0], w_t[sel2, 1]])
        o = np.argsort(toks, kind="stable")
        glist.append(toks[o])
        gw.append(ws[o].astype(np.float32))
    counts = [len(g) for g in glist]
    cap = ((max(counts) + 127) // 128) * 128

    if ("routed", cap) not in _cache:
        _cache[("routed", cap)] = _build_routed(cap)
    nc = _cache[("routed", cap)]

    in_maps = []
    for e in range(NE):
        cnt = counts[e]
        gidx = np.zeros(cap, np.int16)
        gidx[:cnt] = glist[e]
        wvec = np.zeros(cap, np.float32)
        wvec[:cnt] = gw[e]
        # idx table: position i at [i % 16, i // 16], replicated to all 8
        # GPSIMD-core partition groups (HW reads per-core copies).
        idx16 = np.ascontiguousarray(
            np.tile(gidx.reshape(cap // 16, 16).T, (8, 1))
        ).astype(np.int16)
        we_c = np.ascontiguousarray(
            We[e].reshape(KT, 128, DOUT).transpose(1, 0, 2)
        ).reshape(128, KT * DOUT).astype(np.float16)
        ber = np.ascontiguousarray(
            np.broadcast_to(be[e].reshape(1, DOUT), (128, DOUT))
        )
        in_maps.append({"xb": xb, "we": we_c, "ber": ber, "gidx": idx16})

    res = run_bass_kernel_spmd(nc, in_maps, list(range(NCORES)))
    last_results = res

    out_full = np.zeros((T, DOUT), np.float32)
    for e in range(NE):
        cnt = counts[e]
        yg = np.asarray(res.results[e]["yg"][:cnt]).astype(np.float32)
        out_full[glist[e]] += gw[e][:cnt, None] * yg
    return out_full.reshape(NB, NLOC, DOUT)


def _build_routed2(tpc, nprim):
    """Load-balanced expert-parallel program.

    Every core computes `tpc` 128-token tiles: the first `nprim` use the
    core's resident primary-expert weights; the remaining `nov` tiles use
    per-tile weights (+bias) DMA'd from DRAM, letting overloaded experts
    spill whole tiles to under-loaded cores.  Same gather/epilogue as
    _build_routed.
    """
    nov = tpc - nprim
    cap = tpc * 128
    nc = bacc.Bacc("TRN2", target_bir_lowering=False, debug=False)
    xb_d = nc.declare_dram_parameter("xb", [T, DIN], F16, isOutput=False)
    we_d = nc.declare_dram_parameter("we", [128, KT * DOUT], F16, isOutput=False)
    ber_d = nc.declare_dram_parameter("ber", [128, DOUT], F32, isOutput=False)
    wem_d = nc.declare_dram_parameter(
        "wem", [128, nov * KT * DOUT], F16, isOutput=False
    )
    bem_d = nc.declare_dram_parameter("bem", [128, nov * DOUT], F16, isOutput=False)
    gidx_d = nc.declare_dram_parameter("gidx", [128, cap // 16], I16, isOutput=False)
    yg_d = nc.declare_dram_parameter("yg", [cap, DOUT], F16, isOutput=True)

    with TileContext(nc) as tc:
        with (
            tc.tile_pool(name="const", bufs=1) as cpool,
            tc.tile_pool(name="xg", bufs=4) as xgpool,
            tc.tile_pool(name="ow", bufs=3) as opool,
            tc.tile_pool(name="work", bufs=6) as wpool,
            tc.tile_pool(name="psum", bufs=1, space="PSUM") as ppool,
        ):
            idx_sb = cpool.tile([128, cap // 16], I16)
            nc.sync.dma_start(idx_sb[:], gidx_d[:])
            we_sb = cpool.tile([128, KT * DOUT], F16)
            nc.sync.dma_start(we_sb[:], we_d[:])
            ber_sb = cpool.tile([128, DOUT], F32)
            nc.sync.dma_start(ber_sb[:], ber_d[:])

            xgs = {}
            ow = {}
            for m in range(tpc):
                if m % (GCHUNK // 128) == 0:
                    g0 = m * 128
                    glen = min(GCHUNK, cap - g0)
                    xgm = xgpool.tile([128, KT, glen], F16, name="xgm", tag="xgm")
                    nc.gpsimd.dma_gather(
                        out_ap=xgm[:],
                        in_ap=xb_d[:],
                        idxs_ap=idx_sb[:, g0 // 16 : (g0 + glen) // 16],
                        num_idxs=glen,
                        num_idxs_reg=glen,
                        elem_size=DIN,
                        transpose=True,
                    )
                    xgs[m // (GCHUNK // 128)] = xgm
                xgm = xgs[m // (GCHUNK // 128)]
                off = (m % (GCHUNK // 128)) * 128
                if m < nprim:
                    wsrc, bsrc = we_sb, ber_sb[:]
                    woff = 0
                else:
                    # overflow tile: stream this slot's weights+bias from DRAM
                    # (emitted at use site so Tile schedules them just-in-time,
                    # prefetched `bufs` slots ahead, instead of up-front where
                    # they would starve the gathers of SDMA bandwidth)
                    j = m - nprim
                    wj = opool.tile([128, KT * DOUT], F16, name="wj", tag="wj")
                    nc.sync.dma_start(
                        wj[:], wem_d[:, j * KT * DOUT : (j + 1) * KT * DOUT]
                    )
                    bj = opool.tile([128, DOUT], F16, name="bj", tag="bj")
                    nc.sync.dma_start(bj[:], bem_d[:, j * DOUT : (j + 1) * DOUT])
                    wsrc, bsrc = wj, bj[:]
                    woff = 0
                ps = ppool.tile([128, DOUT], F32, name=f"ps{m % 8}", tag=f"ps{m % 8}")
                for k in range(KT):
                    nc.tensor.matmul(
                        ps[:],
                        xgm[:, k, off : off + 128],
                        wsrc[:, woff + k * DOUT : woff + (k + 1) * DOUT],
                        start=(k == 0),
                        stop=(k == KT - 1),
                    )
                t1 = wpool.tile([128, DOUT], F32, tag="t1")
                nc.vector.tensor_add(t1[:], ps[:], bsrc)
                yg = wpool.tile([128, DOUT], F16, tag="yg")
                nc.scalar.activation(yg[:], t1[:], mybir.ActivationFunctionType.Tanh)
                nc.sync.dma_start(yg_d[bass.ts(m, 128), :], yg[:])
    nc.compile()
    return nc


def _plan_balance(counts):
    """Pick (tpc, nprim) and assign each expert's 128-token tiles to cores.

    Returns (tpc, nprim, assign) where assign[c] is a list of length tpc of
    (expert, start, length) pieces ((c, 0, 0)-style dummies have length 0).
    Slot m < nprim must hold expert c (the core's resident expert); slots
    m >= nprim may hold any expert (weights come via the wem input).
    """
    ne = len(counts)
    ceils = [(c + 127) // 128 for c in counts]
    total = sum(ceils)
    tpc0 = max((total + ne - 1) // ne, 1)
    for tpc in range(tpc0, tpc0 + 64):
        nprim = None
        for cand in range(tpc, -1, -1):
            spill = sum(max(ce - cand, 0) for ce in ceils)
            if spill <= ne * (tpc - cand):
                nprim = cand
                break
        if nprim is not None:
            break
    assert nprim is not None
    # primary slots: expert c's first min(ceil_c, nprim) tiles on core c
    assign = []
    spill_tiles = []
    for e in range(ne):
        nown = min(ceils[e], nprim)
        tiles = [(e, t * 128, min(128, counts[e] - t * 128)) for t in range(ceils[e])]
        own = tiles[:nown] + [(e, 0, 0)] * (nprim - nown)
        assign.append(own)
        spill_tiles.extend(tiles[nown:])
    # overflow slots round-robin
    nov = tpc - nprim
    for c in range(ne):
        take, spill_tiles = spill_tiles[:nov], spill_tiles[nov:]
        take = take + [(c, 0, 0)] * (nov - take.__len__())
        assign[c] = assign[c] + take
    assert not spill_tiles
    return tpc, nprim, assign


def _kernel_routed2(x, type_embeddings, atom_types, Wg, We, be):
    global last_results
    x = np.asarray(x, np.float32)
    We = np.asarray(We, np.float32)
    be = np.asarray(be, np.float32)
    _, top2_t, w_t = _routing(
        np.asarray(type_embeddings, np.float32),
        np.asarray(Wg, np.float32),
        np.asarray(atom_types),
    )
    xb = x.reshape(T, DIN).astype(np.float16)

    glist, gw = [], []
    for e in range(NE):
        sel1 = np.nonzero(top2_t[:, 0] == e)[0]
        sel2 = np.nonzero(top2_t[:, 1] == e)[0]
        toks = np.concatenate([sel1, sel2])
        ws = np.concatenate([w_t[sel1, 0], w_t[sel2, 1]])
        o = np.argsort(toks, kind="stable")
        glist.append(toks[o])
        gw.append(ws[o].astype(np.float32))
    counts = [len(g) for g in glist]

    tpc, nprim, assign = _plan_balance(counts)
    nov = tpc - nprim
    cap = tpc * 128
    if ("routed2", tpc, nprim) not in _cache:
        _cache[("routed2", tpc, nprim)] = _build_routed2(tpc, nprim)
    nc = _cache[("routed2", tpc, nprim)]

    we_h = [
        np.ascontiguousarray(We[e].reshape(KT, 128, DOUT).transpose(1, 0, 2))
        .reshape(128, KT * DOUT)
        .astype(np.float16)
        for e in range(NE)
    ]
    ber_h = [
        np.ascontiguousarray(np.broadcast_to(be[e].reshape(1, DOUT), (128, DOUT)))
        for e in range(NE)
    ]
    in_maps = []
    for c in range(NCORES):
        gidx = np.zeros(cap, np.int16)
        for m, (e, s, L) in enumerate(assign[c]):
            if L:
                gidx[m * 128 : m * 128 + L] = glist[e][s : s + L]
        idx16 = np.ascontiguousarray(
            np.tile(gidx.reshape(cap // 16, 16).T, (8, 1))
        ).astype(np.int16)
        wem = np.concatenate(
            [we_h[e] for (e, s, L) in assign[c][nprim:]], axis=1
        ) if nov else np.zeros((128, 0), np.float16)
        bem = np.concatenate(
            [ber_h[e].astype(np.float16) for (e, s, L) in assign[c][nprim:]], axis=1
        ) if nov else np.zeros((128, 0), np.float16)
        in_maps.append(
            {
                "xb": xb,
                "we": we_h[c],
                "ber": ber_h[c],
                "wem": np.ascontiguousarray(wem),
                "bem": np.ascontiguousarray(bem),
                "gidx": idx16,
            }
        )

    res = run_bass_kernel_spmd(nc, in_maps, list(range(NCORES)))
    last_results = res

    out_full = np.zeros((T, DOUT), np.float32)
    # accumulate per expert (each expert's tiles partition its token list,
    # so indices are unique within one fancy-index add)
    for e in range(NE):
        ids, rows, ws = [], [], []
        for c in range(NCORES):
            yg = None
            for m, (te, s, L) in enumerate(assign[c]):
                if te == e and L:
                    if yg is None:
                        yg = np.asarray(res.results[c]["yg"])
                    ids.append(glist[e][s : s + L])
                    rows.append(yg[m * 128 : m * 128 + L])
                    ws.append(gw[e][s : s + L])
        if ids:
            ids = np.concatenate(ids)
            rows = np.concatenate(rows).astype(np.float32)
            ws = np.concatenate(ws)
            out_full[ids] += ws[:, None] * rows
    return out_full.reshape(NB, NLOC, DOUT)


GRAN = 256  # tokens per group (matmul moving dim) in the xp design


def _plan_xp(counts, gran=GRAN):
    """Slot-pattern planner for the transposed expert-parallel design.

    Each core runs C groups of `gran` tokens, partitioned into slots
    (compile-time sizes, identical across cores).  Slot i on core c holds a
    run of groups that all use weight-region i (one expert, per-core data).
    Find (C, sizes, parts) where parts[class_i] = list of (expert, n_groups)
    chunks, such that every expert's ceil(count/gran) groups are covered and
    each class has <= 8 chunks (one per core).
    """
    g = [(c + gran - 1) // gran for c in counts]
    ne = len(g)
    total = sum(g)
    c0 = (total + NCORES - 1) // NCORES

    def decompose(sizes):
        avail = [NCORES] * len(sizes)
        parts = [[] for _ in sizes]
        # every expert gets one largest-class chunk first
        if avail[0] < ne:
            return None
        order = sorted(range(ne), key=lambda e: -g[e])
        for e in order:
            avail[0] -= 1
            parts[0].append((e, min(g[e], sizes[0])))
            r = g[e] - sizes[0]
            while r > 0:
                # largest class <= r with availability, else smallest avail
                pick = None
                for i in range(1, len(sizes)):
                    if avail[i] and sizes[i] <= r:
                        pick = i
                        break
                if pick is None:
                    for i in range(len(sizes) - 1, 0, -1):
                        if avail[i]:
                            pick = i
                            break
                if pick is None:
                    return None
                avail[pick] -= 1
                parts[pick].append((e, min(r, sizes[pick])))
                r -= sizes[pick]
        return parts

    for C in (c0, c0 + 1, c0 + 2):
        patterns = []
        for s1 in range(0, C // 2 + 1):
            for s2 in range(0, s1 + 1):
                s0 = C - s1 - s2
                if s0 >= s1:
                    sizes = [s for s in (s0, s1, s2) if s > 0]
                    patterns.append(tuple(sizes))
        # prefer fewer slots (less weight DMA), then larger primary
        patterns.sort(key=lambda p: (len(p), -p[0]))
        for sizes in patterns:
            parts = decompose(list(sizes))
            if parts is not None:
                return C, list(sizes), parts
    raise RuntimeError("xp plan failed")


N_WARM = 14  # dummy PE warm-up matmuls (pstate ramp) before data arrives


def _build_xp(C, sizes):
    """Transposed expert-parallel program.

    Layout: dout on partitions, tokens on the free (moving) dim.  Per group
    of GRAN tokens: 16 matmuls (4 dout-blocks x 4 k-slices) with stationary
    weight blocks resident in SBUF, then 4 ACT tanh+bias (bias is
    per-partition in this layout) PSUM->SBUF fp16, then batched DMA out.
    xg prefetch on the SP queue, yg writeback on the DVE queue so neither
    blocks the other's sequencer.
    """
    nslots = len(sizes)
    nc = bacc.Bacc("TRN2", target_bir_lowering=False, debug=False)
    xg_d = nc.declare_dram_parameter("xg", [128, C * 4 * GRAN], F16, isOutput=False)
    we_d = nc.declare_dram_parameter("we", [128, nslots * 16 * 128], F16, isOutput=False)
    bias_d = nc.declare_dram_parameter("bias", [128, nslots * 4], F32, isOutput=False)
    yg_d = nc.declare_dram_parameter("yg", [128, C * 4 * GRAN], F16, isOutput=True)

    slot_of = []
    for i, s in enumerate(sizes):
        slot_of += [i] * s

    with TileContext(nc) as tc:
        with (
            tc.tile_pool(name="const", bufs=1) as cpool,
            tc.tile_pool(name="xg", bufs=4) as xpool,
            tc.tile_pool(name="yg", bufs=4) as ypool,
            tc.tile_pool(name="work", bufs=4) as wpool,
            tc.tile_pool(name="psum", bufs=2, space="PSUM") as ppool,
        ):
            # PE warm-up: matmuls on a zeroed tile ramp the pstate while the
            # first DMAs are in flight; a dummy activation preloads the tanh
            # table (1.28us) off the critical path.
            dummy = cpool.tile([128, 128 + GRAN], F16)
            nc.vector.memset(dummy[:], 0)
            dummy2 = cpool.tile([128, 16], F16)
            nc.scalar.activation(
                dummy2[:], dummy[:, 0:16], mybir.ActivationFunctionType.Tanh
            )
            wps = ppool.tile([128, 2, GRAN], F32, name="wps", tag="psA")
            for _ in range(N_WARM):
                nc.tensor.matmul(
                    wps[:, 0, :],
                    dummy[:, 0:128],
                    dummy[:, 128 : 128 + GRAN],
                    start=True,
                    stop=True,
                )

            # head: slot0 b0 weight blocks first (small), then first xg group,
            # then the rest of slot0; later slots stream during slot0 compute.
            # One tile per weight region so coarse tile-granularity dependency
            # tracking never makes slot0 matmuls wait on later slots' DMAs.
            # slot0 weights split per block-pair so pair-A matmuls (the first
            # real PE work) wait only on xg0 + weA, not the whole weight load
            # (the tile framework hoists a psum-pair's waits to its first op).
            xg0 = xpool.tile([128, 2, 4, GRAN], F16, name="xg0", tag="xgA")
            nc.sync.dma_start(xg0[:, 0:1], xg_d[:, 0 : 4 * GRAN])
            weA = cpool.tile([128, 8 * 128], F16)
            nc.sync.dma_start(weA[:], we_d[:, 0 : 8 * 128])
            weB = cpool.tile([128, 8 * 128], F16)
            nc.sync.dma_start(weB[:], we_d[:, 8 * 128 : 16 * 128])
            bias_sb = cpool.tile([128, nslots * 4], F32)
            nc.sync.dma_start(bias_sb[:], bias_d[:])
            wslot = {}
            for i in range(1, nslots):
                wslot[i] = cpool.tile([128, 16 * 128], F16, name=f"we{i}")

            def lhs(i, b, k):
                if i == 0:
                    if b < 2:
                        return weA[:, (b * 4 + k) * 128 : (b * 4 + k + 1) * 128]
                    off = ((b - 2) * 4 + k) * 128
                    return weB[:, off : off + 128]
                off = (b * 4 + k) * 128
                return wslot[i][:, off : off + 128]

            # xg chunks: groups 0 and 1 alone (bandwidth-starved head), then
            # pairs, remainder single
            chunks = [(0, 1), (1, 1)]
            gg = 2
            while gg < C:
                n = min(2, C - gg)
                chunks.append((gg, n))
                gg += n
            # remaining weight slots stream in 4-block pieces between xg
            # prefetches (a full 16-block load would starve the xg stream);
            # bias rides after the first pair-chunk (epilogue slack covers it)
            wlate = {}
            nci = 2
            pieces = [(i, p) for i in range(1, nslots) for p in range(4)]
            while pieces:
                take = 2 if nci == 5 else 1
                wlate[nci], pieces = pieces[:take], pieces[take:]
                nci += 1

            for ci, (g0, n) in enumerate(chunks):
                if ci > 0:
                    xgc = xpool.tile([128, 2, 4, GRAN], F16, name=f"xg{g0}", tag="xgA")
                    nc.sync.dma_start(
                        xgc[:, 0:n],
                        xg_d[:, g0 * 4 * GRAN : (g0 + n) * 4 * GRAN],
                    )
                else:
                    xgc = xg0
                for i, piece in wlate.get(ci, ()):
                    nc.sync.dma_start(
                        wslot[i][:, piece * 4 * 128 : (piece + 1) * 4 * 128],
                        we_d[:, (i * 16 + piece * 4) * 128 : (i * 16 + (piece + 1) * 4) * 128],
                    )

                ygc = ypool.tile([128, n, 4, GRAN], F16, name=f"yg{g0}", tag="ygA")
                last = ci == len(chunks) - 1
                for j in range(n):
                    g = g0 + j
                    i = slot_of[g]
                    xg_j = 0 if ci == 0 else j
                    for pair in range(2):  # blocks (0,1) then (2,3)
                        ps = ppool.tile(
                            [128, 2, GRAN],
                            F32,
                            name=f"ps{'AB'[pair]}",
                            tag=f"ps{'AB'[pair]}",
                        )
                        for h in range(2):
                            b = pair * 2 + h
                            for k in range(4):
                                nc.tensor.matmul(
                                    ps[:, h, :],
                                    lhs(i, b, k),
                                    xgc[:, xg_j, k, :],
                                    start=(k == 0),
                                    stop=(k == 3),
                                )
                        # epilogue split across DVE and ACT so neither engine
                        # saturates: pair A = DVE bias-adds + one wide tanh;
                        # pair B = two narrow biased tanhs on ACT.  The final
                        # group is all-narrow so its last ACT isn't queued
                        # behind a wide op waiting on DVE.
                        if pair == 0 and not last:
                            t1 = wpool.tile([128, 2, GRAN], F16, tag="t1")
                            for h in range(2):
                                b = pair * 2 + h
                                nc.vector.tensor_scalar_add(
                                    t1[:, h, :],
                                    ps[:, h, :],
                                    bias_sb[:, i * 4 + b : i * 4 + b + 1],
                                )
                            nc.scalar.activation(
                                ygc[:, j, pair * 2 : pair * 2 + 2, :],
                                t1[:],
                                mybir.ActivationFunctionType.Tanh,
                            )
                        else:
                            for h in range(2):
                                b = pair * 2 + h
                                nc.scalar.activation(
                                    ygc[:, j, b, :],
                                    ps[:, h, :],
                                    mybir.ActivationFunctionType.Tanh,
                                    bias=bias_sb[:, i * 4 + b : i * 4 + b + 1],
                                )
                    # per-group writeback keeps the output stream spread out
                    gb = g * 4 * GRAN
                    if not last:
                        nc.gpsimd.dma_start(yg_d[:, gb : gb + 4 * GRAN], ygc[:, j])
                    else:
                        # final group: per-pair writebacks on the (idle) SP
                        # HWDGE queue so the drain tail is short
                        nc.sync.dma_start(yg_d[:, gb : gb + 2 * GRAN], ygc[:, j, 0:2])
                        nc.sync.dma_start(
                            yg_d[:, gb + 2 * GRAN : gb + 4 * GRAN], ygc[:, j, 2:4]
                        )
    nc.compile()
    return nc


def _kernel_xp(x, type_embeddings, atom_types, Wg, We, be):
    global last_results
    x = np.asarray(x, np.float32)
    We = np.asarray(We, np.float32)
    be = np.asarray(be, np.float32)
    _, top2_t, w_t = _routing(
        np.asarray(type_embeddings, np.float32),
        np.asarray(Wg, np.float32),
        np.asarray(atom_types),
    )
    x2 = x.reshape(T, DIN)

    glist, gw = [], []
    for e in range(NE):
        sel1 = np.nonzero(top2_t[:, 0] == e)[0]
        sel2 = np.nonzero(top2_t[:, 1] == e)[0]
        toks = np.concatenate([sel1, sel2])
        ws = np.concatenate([w_t[sel1, 0], w_t[sel2, 1]])
        o = np.argsort(toks, kind="stable")
        glist.append(toks[o])
        gw.append(ws[o].astype(np.float32))
    counts = [len(g) for g in glist]

    C, sizes, parts = _plan_xp(counts)
    nslots = len(sizes)
    if ("xp", C, tuple(sizes)) not in _cache:
        _cache[("xp", C, tuple(sizes))] = _build_xp(C, sizes)
    nc = _cache[("xp", C, tuple(sizes))]

    # assign chunks to (core, slot): class i chunk list padded to 8 with
    # dummies; big primary chunks paired with small secondary chunks.
    used = [0] * NE  # groups of expert e already assigned
    asn = [[None] * nslots for _ in range(NCORES)]
    for i in range(nslots):
        chunk_list = list(parts[i]) + [(0, 0)] * (NCORES - len(parts[i]))
        if i == 0:
            chunk_list.sort(key=lambda t: -t[1])
        else:
            chunk_list.sort(key=lambda t: t[1])
        for c in range(NCORES):
            asn[c][i] = chunk_list[c]
    # materialize token ranges in class-major deterministic order
    core_parts = [[] for _ in range(NCORES)]  # (slot, expert, tok_start, n_tok)
    for i in range(nslots):
        for c in range(NCORES):
            e, ngr = asn[c][i]
            tok0 = used[e] * GRAN if ngr else 0
            ntok = min(counts[e] - tok0, ngr * GRAN) if ngr else 0
            ntok = max(ntok, 0)
            if ngr:
                used[e] += ngr
            core_parts[c].append((i, e, tok0, ntok))

    we_h = [
        np.ascontiguousarray(
            We[e].reshape(4, 128, 4, 128).transpose(1, 2, 0, 3)
        ).reshape(128, 16 * 128).astype(np.float16)
        for e in range(NE)
    ]  # [c, b, k, d]
    bias_h = [np.ascontiguousarray(be[e].reshape(4, 128).T) for e in range(NE)]

    in_maps = []
    for c in range(NCORES):
        tok_ids = np.zeros(C * GRAN, np.int64)
        valid = np.zeros(C * GRAN, bool)
        g_base = 0
        we_np = np.zeros((128, nslots * 16 * 128), np.float16)
        bias_np = np.zeros((128, nslots * 4), np.float32)
        for (i, e, tok0, ntok) in core_parts[c]:
            sl0 = g_base * GRAN
            tok_ids[sl0 : sl0 + ntok] = glist[e][tok0 : tok0 + ntok]
            valid[sl0 : sl0 + ntok] = True
            we_np[:, i * 16 * 128 : (i + 1) * 16 * 128] = we_h[e]
            bias_np[:, i * 4 : (i + 1) * 4] = bias_h[e]
            g_base += sizes[i]
        xg = x2[tok_ids].astype(np.float16)
        xg[~valid] = 0
        # [slot(C*GRAN), din] -> [c, g, k, t]
        xg_np = np.ascontiguousarray(
            xg.reshape(C, GRAN, 4, 128).transpose(3, 0, 2, 1)
        ).reshape(128, C * 4 * GRAN)
        in_maps.append(
            {"xg": xg_np, "we": we_np, "bias": bias_np}
        )

    res = run_bass_kernel_spmd(nc, in_maps, list(range(NCORES)))
    last_results = res

    out_full = np.zeros((T, DOUT), np.float32)
    for c in range(NCORES):
        yg = np.asarray(res.results[c]["yg"])
        # [128(d_low), C, 4(b), GRAN] -> rows [C*GRAN, 512]
        rows = (
            yg.reshape(128, C, 4, GRAN)
            .transpose(1, 3, 2, 0)
            .reshape(C * GRAN, DOUT)
            .astype(np.float32)
        )
        g_base = 0
        for (i, e, tok0, ntok) in core_parts[c]:
            sl0 = g_base * GRAN
            if ntok:
                ids = glist[e][tok0 : tok0 + ntok]
                ws = gw[e][tok0 : tok0 + ntok]
                out_full[ids] += ws[:, None] * rows[sl0 : sl0 + ntok]
            g_base += sizes[i]
    return out_full.reshape(NB, NLOC, DOUT)


def kernel(x, type_embeddings, atom_types, Wg, We, be):
    global last_results
    design = os.environ.get("MOE_DESIGN", "routed")
    if design == "xp":
        return _kernel_xp(x, type_embeddings, atom_types, Wg, We, be)
    if design == "routed2":
        return _kernel_routed2(x, type_embeddings, atom_types, Wg, We, be)
    if design == "routed":
        return _kernel_routed(x, type_embeddings, atom_types, Wg, We, be)
    x = np.asarray(x, np.float32)
    We = np.asarray(We, np.float32)
    be = np.asarray(be, np.float32)
    ptw, _, _ = _routing(
        np.asarray(type_embeddings, np.float32),
        np.asarray(Wg, np.float32),
        np.asarray(atom_types),
    )

    x2 = x.reshape(T, DIN)
    ber = np.ascontiguousarray(
        np.broadcast_to(be.reshape(1, NE * DOUT), (128, NE * DOUT))
    )
    # [128, NE*KT*DOUT]: we_h[p, (e*KT+k)*DOUT + d] = We[e, k*128+p, d]
    we_h = np.ascontiguousarray(
        We.reshape(NE, KT, 128, DOUT).transpose(2, 0, 1, 3)
    ).reshape(128, NE * KT * DOUT)
    in_maps = []
    for c in range(NCORES):
        x2c = x2[c * TC : (c + 1) * TC]
        # [128, KT*TC]: xt[p, k*TC + n] = x2c[n, k*128+p]
        xt = np.ascontiguousarray(
            x2c.reshape(TC, KT, 128).transpose(2, 1, 0)
        ).reshape(128, KT * TC)
        xw = np.concatenate([xt, we_h], axis=1)
        pwl = np.ascontiguousarray(
            ptw[c * TC : (c + 1) * TC].reshape(MT, 128, NE).transpose(1, 0, 2)
        ).reshape(128, MT * NE)
        in_maps.append({"xw": xw, "pwl": pwl, "ber": ber})

    if "dense" not in _cache:
        _cache["dense"] = _build_dense()
    nc = _cache["dense"]

    res = run_bass_kernel_spmd(nc, in_maps, list(range(NCORES)))
    last_results = res
    out = np.concatenate([res.results[c]["out"] for c in range(NCORES)], axis=0)
    return out.reshape(NB, NLOC, DOUT).astype(np.float32)



# revision 32
# speedup vs baseline: 1.3359x; 1.0292x over previous
"""MoE layer (routed top-2 of 8 experts) on 8 TRN2 NeuronCores.

Contract: kernel(**inputs) takes the FULL unsharded inputs and returns the
FULL [4, 4096, 512] float32 output. Sharding/compile/run happens inside.

Primary design (MOE_DESIGN=xp, default) - transposed expert-parallel:
  - Host computes the gating, pre-gathers/transposes each core's tokens into
    the matmul moving layout [din-part, group, k, token] fp16 (no on-device
    gathers), and packs per-core "weight slot" regions so one SPMD program
    serves all cores: each core runs C groups of GRAN tokens split into
    compile-time slots; slot i's weights/bias are per-core param data.
  - Device (per group): 16 matmuls (4 dout-blocks x 4 k) with dout on PSUM
    partitions, tokens on the moving dim; epilogue split DVE/ACT (pair A:
    DVE bias-add + wide tanh, pair B: biased narrow tanhs; bias is per-
    partition in this layout); per-group writeback on the Pool SWDGE queue,
    xg prefetch on the SP HWDGE queue; PE-pstate warm-up matmuls at t=0.
  - Host applies gate weights and scatter-adds yg rows into the output.

Fallback design (expert-parallel gather-based, MOE_DESIGN=routed):
  - Host computes the gating (128x128 @ 128x8 matmul, top-2, softmax) and
    per-expert token index lists.
  - Core e owns expert e: it gathers its expert's tokens directly from the
    full fp16 x in DRAM with a transposed dma_gather (tokens land as
    [din, token] tiles, exactly the matmul stationary layout), runs the
    512x512 expert matmul in fp16 with fp32 PSUM accumulation, adds the
    bias (DVE) and applies tanh (ACT, casting to fp16), and streams the
    dense gathered result yg = tanh(x[idx] @ We + be) back to DRAM.
  - Host applies the gate weights and scatter-adds each core's yg rows into
    the final fp32 output (each core's row set is duplicate-free, so this
    is a plain fancy-index add).

Fallback (MOE_DESIGN=dense): data-parallel dense-expert kernel in float32r
(~4x slower, ~2e-4 more accurate).
"""

import os

import numpy as np

# The axon NTFF profiling hooks (antenv.axon_hooks) are not shipped in this
# container; BASS_TRACE=1 in the environment would crash run_bass_kernel_spmd.
os.environ["BASS_NEVER_TRACE"] = "1"

import concourse.bass as bass
import concourse.bacc as bacc
import concourse.mybir as mybir
from concourse.tile import TileContext
from concourse.bass_utils import run_bass_kernel_spmd

F32 = mybir.dt.float32
F32R = mybir.dt.float32r
BF16 = mybir.dt.bfloat16
I16 = mybir.dt.int16
F16 = mybir.dt.float16

NB, NLOC, DIN, DOUT = 4, 4096, 512, 512
NTYPES, TEBD = 128, 128
NE, TOPK = 8, 2
NCORES = 8
T = NB * NLOC          # 16384 tokens
TC = T // NCORES       # 2048 tokens per core
MT = TC // 128         # 16 token m-tiles per core
KT = DIN // 128        # 4 k-tiles

_cache = {}

# set by run (module-level so test.py can read timing/trace results)
last_results = None


def _routing(type_embeddings, Wg, atom_types):
    """Host-side gating math (tiny): per-token dense expert weights [T, NE]."""
    logits = type_embeddings.astype(np.float32) @ Wg.astype(np.float32)  # [NTYPES, NE]
    order = np.argsort(-logits, axis=-1, kind="stable")                  # stable => ties to lower idx
    top2 = order[:, :TOPK]                                               # [NTYPES, 2]
    tv = np.take_along_axis(logits, top2, axis=-1)                       # [NTYPES, 2]
    ex = np.exp(tv - tv.max(axis=-1, keepdims=True))
    w = (ex / ex.sum(axis=-1, keepdims=True)).astype(np.float32)         # [NTYPES, 2]
    ptw_types = np.zeros((NTYPES, NE), np.float32)
    np.put_along_axis(ptw_types, top2, w, axis=-1)
    at = atom_types.reshape(-1)
    return ptw_types[at], top2[at], w[at]  # dense [T, NE], idx [T,2], w [T,2]


def _build_dense():
    """Dense data-parallel program: out[t,:] = sum_e pw[t,e]*tanh(x[t]@We[e]+be[e])."""
    nc = bacc.Bacc("TRN2", target_bir_lowering=False, debug=False)
    # xt and We are packed into ONE param/DMA so matmuls have a single
    # producer semaphore (the fp32r LW matmul only fits one sync wait).
    xw_d = nc.declare_dram_parameter("xw", [128, KT * TC + NE * KT * DOUT], F32R, isOutput=False)
    pwl_d = nc.declare_dram_parameter("pwl", [128, MT * NE], F32, isOutput=False)
    ber_d = nc.declare_dram_parameter("ber", [128, NE * DOUT], F32, isOutput=False)
    out_d = nc.declare_dram_parameter("out", [TC, DOUT], F32, isOutput=True)

    with TileContext(nc) as tc:
        with (
            tc.tile_pool(name="const", bufs=1) as cpool,
            tc.tile_pool(name="work", bufs=4) as wpool,
            tc.tile_pool(name="psum", bufs=1, space="PSUM") as ppool,
        ):
            xw_sb = cpool.tile([128, KT * TC + NE * KT * DOUT], F32R)
            nc.sync.dma_start(xw_sb[:], xw_d[:])
            XOFF = KT * TC
            pwl_sb = cpool.tile([128, MT * NE], F32)
            nc.sync.dma_start(pwl_sb[:], pwl_d[:])
            ber_sb = cpool.tile([128, NE * DOUT], F32)
            nc.sync.dma_start(ber_sb[:], ber_d[:])

            for m in range(MT):
                pss = []
                for e in range(NE):
                    pss.append(
                        ppool.tile([128, DOUT], F32, name=f"ps{e}", tag=f"ps{e}")
                    )
                for k in range(KT):
                    lhs = xw_sb[:, k * TC + m * 128 : k * TC + (m + 1) * 128]
                    for e in range(NE):
                        nc.tensor.matmul(
                            pss[e][:],
                            lhs,
                            xw_sb[:, XOFF + (e * KT + k) * DOUT : XOFF + (e * KT + k + 1) * DOUT],
                            start=(k == 0),
                            stop=(k == KT - 1),
                        )
                acc = wpool.tile([128, DOUT], F32, tag="acc")
                for e in range(NE):
                    t1 = wpool.tile([128, DOUT], F32, tag="t1")
                    nc.vector.tensor_add(t1[:], pss[e][:], ber_sb[:, bass.ts(e, DOUT)])
                    t2 = wpool.tile([128, DOUT], F32, tag="t2")
                    nc.scalar.activation(
                        t2[:], t1[:], mybir.ActivationFunctionType.Tanh
                    )
                    wsc = pwl_sb[:, m * NE + e : m * NE + e + 1]
                    if e == 0:
                        nc.vector.tensor_scalar_mul(acc[:], t2[:], wsc)
                    else:
                        nc.vector.scalar_tensor_tensor(
                            acc[:],
                            t2[:],
                            wsc,
                            acc[:],
                            op0=mybir.AluOpType.mult,
                            op1=mybir.AluOpType.add,
                        )
                nc.sync.dma_start(out_d[bass.ts(m, 128), :], acc[:])
    nc.compile()
    return nc


GCHUNK = 256  # tokens per dma_gather (balances ~1us SWDGE fixed cost vs pipelining)


def _build_routed(cap):
    """Expert-parallel routed program (one expert per core).

    Each core gathers its expert's tokens from the full fp16 x in DRAM via
    transposed dma_gather ([din, token] tiles), runs the expert matmul in
    fp16 (fp32 accumulate), applies bias+tanh, and writes the dense gathered
    output yg = tanh(x@We+be) [cap, 512] fp16.  The per-token gate weight and
    the scatter-add into the final output happen on host (row sets are unique
    per core, so it is a plain fancy-index add).
    """
    mte = cap // 128
    nc = bacc.Bacc("TRN2", target_bir_lowering=False, debug=False)
    xb_d = nc.declare_dram_parameter("xb", [T, DIN], F16, isOutput=False)
    we_d = nc.declare_dram_parameter("we", [128, KT * DOUT], F16, isOutput=False)
    ber_d = nc.declare_dram_parameter("ber", [128, DOUT], F32, isOutput=False)
    gidx_d = nc.declare_dram_parameter("gidx", [128, cap // 16], I16, isOutput=False)
    yg_d = nc.declare_dram_parameter("yg", [cap, DOUT], F16, isOutput=True)

    with TileContext(nc) as tc:
        with (
            tc.tile_pool(name="const", bufs=1) as cpool,
            tc.tile_pool(name="xg", bufs=4) as xgpool,
            tc.tile_pool(name="work", bufs=6) as wpool,
            tc.tile_pool(name="psum", bufs=1, space="PSUM") as ppool,
        ):
            # idx first: HWDGE DMAs drain FIFO per engine, and the gathers
            # (critical-path head) wait on the index table.
            idx_sb = cpool.tile([128, cap // 16], I16)
            nc.sync.dma_start(idx_sb[:], gidx_d[:])
            we_sb = cpool.tile([128, KT * DOUT], F16)
            nc.sync.dma_start(we_sb[:], we_d[:])
            ber_sb = cpool.tile([128, DOUT], F32)
            nc.sync.dma_start(ber_sb[:], ber_d[:])

            # Chunk pattern: two small 128-token warm-up gathers so the PE
            # starts ~2us sooner, then 256-token chunks (the measured HW
            # optimum), remainder last.
            chunks = [128, 128] if cap > 256 else [cap]
            rest = cap - sum(chunks)
            chunks += [GCHUNK] * (rest // GCHUNK)
            if rest % GCHUNK:
                chunks.append(rest % GCHUNK)
            g0 = 0
            m = 0
            for glen in chunks:
                xgm = xgpool.tile([128, KT, glen], F16, name="xgm", tag="xgm")
                nc.gpsimd.dma_gather(
                    out_ap=xgm[:],
                    in_ap=xb_d[:],
                    idxs_ap=idx_sb[:, g0 // 16 : (g0 + glen) // 16],
                    num_idxs=glen,
                    num_idxs_reg=glen,
                    elem_size=DIN,
                    transpose=True,
                )
                for off in range(0, glen, 128):
                    ps = ppool.tile(
                        [128, DOUT], F32, name=f"ps{m % 8}", tag=f"ps{m % 8}"
                    )
                    for k in range(KT):
                        nc.tensor.matmul(
                            ps[:],
                            xgm[:, k, off : off + 128],
                            we_sb[:, bass.ts(k, DOUT)],
                            start=(k == 0),
                            stop=(k == KT - 1),
                        )
                    t1 = wpool.tile([128, DOUT], F32, tag="t1")
                    nc.vector.tensor_add(t1[:], ps[:], ber_sb[:])
                    yg = wpool.tile([128, DOUT], F16, tag="yg")
                    nc.scalar.activation(
                        yg[:], t1[:], mybir.ActivationFunctionType.Tanh
                    )
                    nc.sync.dma_start(yg_d[bass.ts(m, 128), :], yg[:])
                    m += 1
                g0 += glen
    nc.compile()
    return nc


def _kernel_routed(x, type_embeddings, atom_types, Wg, We, be):
    global last_results
    x = np.asarray(x, np.float32)
    We = np.asarray(We, np.float32)
    be = np.asarray(be, np.float32)
    _, top2_t, w_t = _routing(
        np.asarray(type_embeddings, np.float32),
        np.asarray(Wg, np.float32),
        np.asarray(atom_types),
    )  # top2_t [T,2], w_t [T,2]

    x2 = x.reshape(T, DIN)
    xb = x2.astype(np.float16)

    # per-expert token lists (ascending token order)
    glist, gw = [], []
    for e in range(NE):
        sel1 = np.nonzero(top2_t[:, 0] == e)[0]
        sel2 = np.nonzero(top2_t[:, 1] == e)[0]
        toks = np.concatenate([sel1, sel2])
        ws = np.concatenate([w_t[sel1, 0], w_t[sel2, 1]])
        o = np.argsort(toks, kind="stable")
        glist.append(toks[o])
        gw.append(ws[o].astype(np.float32))
    counts = [len(g) for g in glist]
    cap = ((max(counts) + 127) // 128) * 128

    if ("routed", cap) not in _cache:
        _cache[("routed", cap)] = _build_routed(cap)
    nc = _cache[("routed", cap)]

    in_maps = []
    for e in range(NE):
        cnt = counts[e]
        gidx = np.zeros(cap, np.int16)
        gidx[:cnt] = glist[e]
        wvec = np.zeros(cap, np.float32)
        wvec[:cnt] = gw[e]
        # idx table: position i at [i % 16, i // 16], replicated to all 8
        # GPSIMD-core partition groups (HW reads per-core copies).
        idx16 = np.ascontiguousarray(
            np.tile(gidx.reshape(cap // 16, 16).T, (8, 1))
        ).astype(np.int16)
        we_c = np.ascontiguousarray(
            We[e].reshape(KT, 128, DOUT).transpose(1, 0, 2)
        ).reshape(128, KT * DOUT).astype(np.float16)
        ber = np.ascontiguousarray(
            np.broadcast_to(be[e].reshape(1, DOUT), (128, DOUT))
        )
        in_maps.append({"xb": xb, "we": we_c, "ber": ber, "gidx": idx16})

    res = run_bass_kernel_spmd(nc, in_maps, list(range(NCORES)))
    last_results = res

    out_full = np.zeros((T, DOUT), np.float32)
    for e in range(NE):
        cnt = counts[e]
        yg = np.asarray(res.results[e]["yg"][:cnt]).astype(np.float32)
        out_full[glist[e]] += gw[e][:cnt, None] * yg
    return out_full.reshape(NB, NLOC, DOUT)


def _build_routed2(tpc, nprim):
    """Load-balanced expert-parallel program.

    Every core computes `tpc` 128-token tiles: the first `nprim` use the
    core's resident primary-expert weights; the remaining `nov` tiles use
    per-tile weights (+bias) DMA'd from DRAM, letting overloaded experts
    spill whole tiles to under-loaded cores.  Same gather/epilogue as
    _build_routed.
    """
    nov = tpc - nprim
    cap = tpc * 128
    nc = bacc.Bacc("TRN2", target_bir_lowering=False, debug=False)
    xb_d = nc.declare_dram_parameter("xb", [T, DIN], F16, isOutput=False)
    we_d = nc.declare_dram_parameter("we", [128, KT * DOUT], F16, isOutput=False)
    ber_d = nc.declare_dram_parameter("ber", [128, DOUT], F32, isOutput=False)
    wem_d = nc.declare_dram_parameter(
        "wem", [128, nov * KT * DOUT], F16, isOutput=False
    )
    bem_d = nc.declare_dram_parameter("bem", [128, nov * DOUT], F16, isOutput=False)
    gidx_d = nc.declare_dram_parameter("gidx", [128, cap // 16], I16, isOutput=False)
    yg_d = nc.declare_dram_parameter("yg", [cap, DOUT], F16, isOutput=True)

    with TileContext(nc) as tc:
        with (
            tc.tile_pool(name="const", bufs=1) as cpool,
            tc.tile_pool(name="xg", bufs=4) as xgpool,
            tc.tile_pool(name="ow", bufs=3) as opool,
            tc.tile_pool(name="work", bufs=6) as wpool,
            tc.tile_pool(name="psum", bufs=1, space="PSUM") as ppool,
        ):
            idx_sb = cpool.tile([128, cap // 16], I16)
            nc.sync.dma_start(idx_sb[:], gidx_d[:])
            we_sb = cpool.tile([128, KT * DOUT], F16)
            nc.sync.dma_start(we_sb[:], we_d[:])
            ber_sb = cpool.tile([128, DOUT], F32)
            nc.sync.dma_start(ber_sb[:], ber_d[:])

            xgs = {}
            ow = {}
            for m in range(tpc):
                if m % (GCHUNK // 128) == 0:
                    g0 = m * 128
                    glen = min(GCHUNK, cap - g0)
                    xgm = xgpool.tile([128, KT, glen], F16, name="xgm", tag="xgm")
                    nc.gpsimd.dma_gather(
                        out_ap=xgm[:],
                        in_ap=xb_d[:],
                        idxs_ap=idx_sb[:, g0 // 16 : (g0 + glen) // 16],
                        num_idxs=glen,
                        num_idxs_reg=glen,
                        elem_size=DIN,
                        transpose=True,
                    )
                    xgs[m // (GCHUNK // 128)] = xgm
                xgm = xgs[m // (GCHUNK // 128)]
                off = (m % (GCHUNK // 128)) * 128
                if m < nprim:
                    wsrc, bsrc = we_sb, ber_sb[:]
                    woff = 0
                else:
                    # overflow tile: stream this slot's weights+bias from DRAM
                    # (emitted at use site so Tile schedules them just-in-time,
                    # prefetched `bufs` slots ahead, instead of up-front where
                    # they would starve the gathers of SDMA bandwidth)
                    j = m - nprim
                    wj = opool.tile([128, KT * DOUT], F16, name="wj", tag="wj")
                    nc.sync.dma_start(
                        wj[:], wem_d[:, j * KT * DOUT : (j + 1) * KT * DOUT]
                    )
                    bj = opool.tile([128, DOUT], F16, name="bj", tag="bj")
                    nc.sync.dma_start(bj[:], bem_d[:, j * DOUT : (j + 1) * DOUT])
                    wsrc, bsrc = wj, bj[:]
                    woff = 0
                ps = ppool.tile([128, DOUT], F32, name=f"ps{m % 8}", tag=f"ps{m % 8}")
                for k in range(KT):
                    nc.tensor.matmul(
                        ps[:],
                        xgm[:, k, off : off + 128],
                        wsrc[:, woff + k * DOUT : woff + (k + 1) * DOUT],
                        start=(k == 0),
                        stop=(k == KT - 1),
                    )
                t1 = wpool.tile([128, DOUT], F32, tag="t1")
                nc.vector.tensor_add(t1[:], ps[:], bsrc)
                yg = wpool.tile([128, DOUT], F16, tag="yg")
                nc.scalar.activation(yg[:], t1[:], mybir.ActivationFunctionType.Tanh)
                nc.sync.dma_start(yg_d[bass.ts(m, 128), :], yg[:])
    nc.compile()
    return nc


def _plan_balance(counts):
    """Pick (tpc, nprim) and assign each expert's 128-token tiles to cores.

    Returns (tpc, nprim, assign) where assign[c] is a list of length tpc of
    (expert, start, length) pieces ((c, 0, 0)-style dummies have length 0).
    Slot m < nprim must hold expert c (the core's resident expert); slots
    m >= nprim may hold any expert (weights come via the wem input).
    """
    ne = len(counts)
    ceils = [(c + 127) // 128 for c in counts]
    total = sum(ceils)
    tpc0 = max((total + ne - 1) // ne, 1)
    for tpc in range(tpc0, tpc0 + 64):
        nprim = None
        for cand in range(tpc, -1, -1):
            spill = sum(max(ce - cand, 0) for ce in ceils)
            if spill <= ne * (tpc - cand):
                nprim = cand
                break
        if nprim is not None:
            break
    assert nprim is not None
    # primary slots: expert c's first min(ceil_c, nprim) tiles on core c
    assign = []
    spill_tiles = []
    for e in range(ne):
        nown = min(ceils[e], nprim)
        tiles = [(e, t * 128, min(128, counts[e] - t * 128)) for t in range(ceils[e])]
        own = tiles[:nown] + [(e, 0, 0)] * (nprim - nown)
        assign.append(own)
        spill_tiles.extend(tiles[nown:])
    # overflow slots round-robin
    nov = tpc - nprim
    for c in range(ne):
        take, spill_tiles = spill_tiles[:nov], spill_tiles[nov:]
        take = take + [(c, 0, 0)] * (nov - take.__len__())
        assign[c] = assign[c] + take
    assert not spill_tiles
    return tpc, nprim, assign


def _kernel_routed2(x, type_embeddings, atom_types, Wg, We, be):
    global last_results
    x = np.asarray(x, np.float32)
    We = np.asarray(We, np.float32)
    be = np.asarray(be, np.float32)
    _, top2_t, w_t = _routing(
        np.asarray(type_embeddings, np.float32),
        np.asarray(Wg, np.float32),
        np.asarray(atom_types),
    )
    xb = x.reshape(T, DIN).astype(np.float16)

    glist, gw = [], []
    for e in range(NE):
        sel1 = np.nonzero(top2_t[:, 0] == e)[0]
        sel2 = np.nonzero(top2_t[:, 1] == e)[0]
        toks = np.concatenate([sel1, sel2])
        ws = np.concatenate([w_t[sel1, 0], w_t[sel2, 1]])
        o = np.argsort(toks, kind="stable")
        glist.append(toks[o])
        gw.append(ws[o].astype(np.float32))
    counts = [len(g) for g in glist]

    tpc, nprim, assign = _plan_balance(counts)
    nov = tpc - nprim
    cap = tpc * 128
    if ("routed2", tpc, nprim) not in _cache:
        _cache[("routed2", tpc, nprim)] = _build_routed2(tpc, nprim)
    nc = _cache[("routed2", tpc, nprim)]

    we_h = [
        np.ascontiguousarray(We[e].reshape(KT, 128, DOUT).transpose(1, 0, 2))
        .reshape(128, KT * DOUT)
        .astype(np.float16)
        for e in range(NE)
    ]
    ber_h = [
        np.ascontiguousarray(np.broadcast_to(be[e].reshape(1, DOUT), (128, DOUT)))
        for e in range(NE)
    ]
    in_maps = []
    for c in range(NCORES):
        gidx = np.zeros(cap, np.int16)
        for m, (e, s, L) in enumerate(assign[c]):
            if L:
                gidx[m * 128 : m * 128 + L] = glist[e][s : s + L]
        idx16 = np.ascontiguousarray(
            np.tile(gidx.reshape(cap // 16, 16).T, (8, 1))
        ).astype(np.int16)
        wem = np.concatenate(
            [we_h[e] for (e, s, L) in assign[c][nprim:]], axis=1
        ) if nov else np.zeros((128, 0), np.float16)
        bem = np.concatenate(
            [ber_h[e].astype(np.float16) for (e, s, L) in assign[c][nprim:]], axis=1
        ) if nov else np.zeros((128, 0), np.float16)
        in_maps.append(
            {
                "xb": xb,
                "we": we_h[c],
                "ber": ber_h[c],
                "wem": np.ascontiguousarray(wem),
                "bem": np.ascontiguousarray(bem),
                "gidx": idx16,
            }
        )

    res = run_bass_kernel_spmd(nc, in_maps, list(range(NCORES)))
    last_results = res

    out_full = np.zeros((T, DOUT), np.float32)
    # accumulate per expert (each expert's tiles partition its token list,
    # so indices are unique within one fancy-index add)
    for e in range(NE):
        ids, rows, ws = [], [], []
        for c in range(NCORES):
            yg = None
            for m, (te, s, L) in enumerate(assign[c]):
                if te == e and L:
                    if yg is None:
                        yg = np.asarray(res.results[c]["yg"])
                    ids.append(glist[e][s : s + L])
                    rows.append(yg[m * 128 : m * 128 + L])
                    ws.append(gw[e][s : s + L])
        if ids:
            ids = np.concatenate(ids)
            rows = np.concatenate(rows).astype(np.float32)
            ws = np.concatenate(ws)
            out_full[ids] += ws[:, None] * rows
    return out_full.reshape(NB, NLOC, DOUT)


GRAN = int(os.environ.get("XP_GRAN", "224"))  # tokens per group (matmul moving dim)


def _plan_xp(counts, gran=GRAN):
    """Slot-pattern planner for the transposed expert-parallel design.

    Each core runs C groups of `gran` tokens, partitioned into slots
    (compile-time sizes, identical across cores).  Slot i on core c holds a
    run of groups that all use weight-region i (one expert, per-core data).
    Find (C, sizes, parts) where parts[class_i] = list of (expert, n_groups)
    chunks, such that every expert's ceil(count/gran) groups are covered and
    each class has <= 8 chunks (one per core).
    """
    g = [(c + gran - 1) // gran for c in counts]
    ne = len(g)
    total = sum(g)
    c0 = (total + NCORES - 1) // NCORES

    def decompose(sizes):
        avail = [NCORES] * len(sizes)
        parts = [[] for _ in sizes]
        # every expert gets one largest-class chunk first
        if avail[0] < ne:
            return None
        order = sorted(range(ne), key=lambda e: -g[e])
        for e in order:
            avail[0] -= 1
            parts[0].append((e, min(g[e], sizes[0])))
            r = g[e] - sizes[0]
            while r > 0:
                # largest class <= r with availability, else smallest avail
                pick = None
                for i in range(1, len(sizes)):
                    if avail[i] and sizes[i] <= r:
                        pick = i
                        break
                if pick is None:
                    for i in range(len(sizes) - 1, 0, -1):
                        if avail[i]:
                            pick = i
                            break
                if pick is None:
                    return None
                avail[pick] -= 1
                parts[pick].append((e, min(r, sizes[pick])))
                r -= sizes[pick]
        return parts

    for C in (c0, c0 + 1, c0 + 2):
        patterns = []
        for s1 in range(0, C // 2 + 1):
            for s2 in range(0, s1 + 1):
                s0 = C - s1 - s2
                if s0 >= s1:
                    sizes = [s for s in (s0, s1, s2) if s > 0]
                    patterns.append(tuple(sizes))
        # prefer fewer slots (less weight DMA), then larger primary
        patterns.sort(key=lambda p: (len(p), -p[0]))
        for sizes in patterns:
            parts = decompose(list(sizes))
            if parts is not None:
                return C, list(sizes), parts
    raise RuntimeError("xp plan failed")


N_WARM = int(os.environ.get("XP_WARM", "14"))  # dummy PE warm-up matmuls (pstate ramp) before data arrives


def _build_xp(C, sizes):
    """Transposed expert-parallel program.

    Layout: dout on partitions, tokens on the free (moving) dim.  Per group
    of GRAN tokens: 16 matmuls (4 dout-blocks x 4 k-slices) with stationary
    weight blocks resident in SBUF, then 4 ACT tanh+bias (bias is
    per-partition in this layout) PSUM->SBUF fp16, then batched DMA out.
    xg prefetch on the SP queue, yg writeback on the DVE queue so neither
    blocks the other's sequencer.
    """
    nslots = len(sizes)
    nc = bacc.Bacc("TRN2", target_bir_lowering=False, debug=False)
    xg_d = nc.declare_dram_parameter("xg", [128, C * 4 * GRAN], F16, isOutput=False)
    we_d = nc.declare_dram_parameter("we", [128, nslots * 16 * 128], F16, isOutput=False)
    bias_d = nc.declare_dram_parameter("bias", [128, nslots * 4], F32, isOutput=False)
    yg_d = nc.declare_dram_parameter("yg", [128, C * 4 * GRAN], F16, isOutput=True)

    slot_of = []
    for i, s in enumerate(sizes):
        slot_of += [i] * s

    with TileContext(nc) as tc:
        with (
            tc.tile_pool(name="const", bufs=1) as cpool,
            tc.tile_pool(name="xg", bufs=4) as xpool,
            tc.tile_pool(name="yg", bufs=4) as ypool,
            tc.tile_pool(name="work", bufs=4) as wpool,
            tc.tile_pool(name="psum", bufs=2, space="PSUM") as ppool,
        ):
            # PE warm-up: matmuls on a zeroed tile ramp the pstate while the
            # first DMAs are in flight; a dummy activation preloads the tanh
            # table (1.28us) off the critical path.
            dummy = cpool.tile([128, 128 + GRAN], F16)
            nc.vector.memset(dummy[:], 0)
            dummy2 = cpool.tile([128, 16], F16)
            nc.scalar.activation(
                dummy2[:], dummy[:, 0:16], mybir.ActivationFunctionType.Tanh
            )
            wps = ppool.tile([128, 2, GRAN], F32, name="wps", tag="psA")
            for _ in range(N_WARM):
                nc.tensor.matmul(
                    wps[:, 0, :],
                    dummy[:, 0:128],
                    dummy[:, 128 : 128 + GRAN],
                    start=True,
                    stop=True,
                )

            # head: slot0 b0 weight blocks first (small), then first xg group,
            # then the rest of slot0; later slots stream during slot0 compute.
            # One tile per weight region so coarse tile-granularity dependency
            # tracking never makes slot0 matmuls wait on later slots' DMAs.
            # slot0 weights split per block-pair so pair-A matmuls (the first
            # real PE work) wait only on xg0 + weA, not the whole weight load
            # (the tile framework hoists a psum-pair's waits to its first op).
            xg0 = xpool.tile([128, 2, 4, GRAN], F16, name="xg0", tag="xgA")
            nc.sync.dma_start(xg0[:, 0:1], xg_d[:, 0 : 4 * GRAN])
            weA = cpool.tile([128, 8 * 128], F16)
            nc.sync.dma_start(weA[:], we_d[:, 0 : 8 * 128])
            weB = cpool.tile([128, 8 * 128], F16)
            nc.sync.dma_start(weB[:], we_d[:, 8 * 128 : 16 * 128])
            bias_sb = cpool.tile([128, nslots * 4], F32)
            nc.sync.dma_start(bias_sb[:], bias_d[:])
            wslot = {}
            for i in range(1, nslots):
                wslot[i] = cpool.tile([128, 16 * 128], F16, name=f"we{i}")

            def lhs(i, b, k):
                if i == 0:
                    if b < 2:
                        return weA[:, (b * 4 + k) * 128 : (b * 4 + k + 1) * 128]
                    off = ((b - 2) * 4 + k) * 128
                    return weB[:, off : off + 128]
                off = (b * 4 + k) * 128
                return wslot[i][:, off : off + 128]

            # xg chunks: groups 0 and 1 alone (bandwidth-starved head), then
            # pairs, remainder single
            chunks = [(0, 1), (1, 1)]
            gg = 2
            while gg < C:
                n = min(2, C - gg)
                chunks.append((gg, n))
                gg += n
            # remaining weight slots stream in 4-block pieces between xg
            # prefetches (a full 16-block load would starve the xg stream);
            # bias rides after the first pair-chunk (epilogue slack covers it)
            wlate = {}
            nci = 2
            pieces = [(i, p) for i in range(1, nslots) for p in range(4)]
            while pieces:
                take = 2 if nci == 5 else 1
                wlate[nci], pieces = pieces[:take], pieces[take:]
                nci += 1

            for ci, (g0, n) in enumerate(chunks):
                if ci > 0:
                    xgc = xpool.tile([128, 2, 4, GRAN], F16, name=f"xg{g0}", tag="xgA")
                    nc.sync.dma_start(
                        xgc[:, 0:n],
                        xg_d[:, g0 * 4 * GRAN : (g0 + n) * 4 * GRAN],
                    )
                else:
                    xgc = xg0
                for i, piece in wlate.get(ci, ()):
                    nc.sync.dma_start(
                        wslot[i][:, piece * 4 * 128 : (piece + 1) * 4 * 128],
                        we_d[:, (i * 16 + piece * 4) * 128 : (i * 16 + (piece + 1) * 4) * 128],
                    )

                ygc = ypool.tile([128, n, 4, GRAN], F16, name=f"yg{g0}", tag="ygA")
                last = ci == len(chunks) - 1
                for j in range(n):
                    g = g0 + j
                    i = slot_of[g]
                    xg_j = 0 if ci == 0 else j
                    for pair in range(2):  # blocks (0,1) then (2,3)
                        ps = ppool.tile(
                            [128, 2, GRAN],
                            F32,
                            name=f"ps{'AB'[pair]}",
                            tag=f"ps{'AB'[pair]}",
                        )
                        for h in range(2):
                            b = pair * 2 + h
                            for k in range(4):
                                nc.tensor.matmul(
                                    ps[:, h, :],
                                    lhs(i, b, k),
                                    xgc[:, xg_j, k, :],
                                    start=(k == 0),
                                    stop=(k == 3),
                                )
                        # epilogue split across DVE and ACT so neither engine
                        # saturates: pair A = DVE bias-adds + one wide tanh;
                        # pair B = two narrow biased tanhs on ACT.  The final
                        # group is all-narrow so its last ACT isn't queued
                        # behind a wide op waiting on DVE.
                        if pair == 0 and not last:
                            t1 = wpool.tile([128, 2, GRAN], F16, tag="t1")
                            for h in range(2):
                                b = pair * 2 + h
                                nc.vector.tensor_scalar_add(
                                    t1[:, h, :],
                                    ps[:, h, :],
                                    bias_sb[:, i * 4 + b : i * 4 + b + 1],
                                )
                            nc.scalar.activation(
                                ygc[:, j, pair * 2 : pair * 2 + 2, :],
                                t1[:],
                                mybir.ActivationFunctionType.Tanh,
                            )
                        else:
                            for h in range(2):
                                b = pair * 2 + h
                                nc.scalar.activation(
                                    ygc[:, j, b, :],
                                    ps[:, h, :],
                                    mybir.ActivationFunctionType.Tanh,
                                    bias=bias_sb[:, i * 4 + b : i * 4 + b + 1],
                                )
                    # per-group writeback keeps the output stream spread out
                    gb = g * 4 * GRAN
                    if not last:
                        nc.gpsimd.dma_start(yg_d[:, gb : gb + 4 * GRAN], ygc[:, j])
                    else:
                        # final group: per-pair writebacks on the (idle) SP
                        # HWDGE queue so the drain tail is short
                        nc.sync.dma_start(yg_d[:, gb : gb + 2 * GRAN], ygc[:, j, 0:2])
                        nc.sync.dma_start(
                            yg_d[:, gb + 2 * GRAN : gb + 4 * GRAN], ygc[:, j, 2:4]
                        )
    nc.compile()
    return nc


def _kernel_xp(x, type_embeddings, atom_types, Wg, We, be):
    global last_results
    x = np.asarray(x, np.float32)
    We = np.asarray(We, np.float32)
    be = np.asarray(be, np.float32)
    _, top2_t, w_t = _routing(
        np.asarray(type_embeddings, np.float32),
        np.asarray(Wg, np.float32),
        np.asarray(atom_types),
    )
    x2 = x.reshape(T, DIN)

    glist, gw = [], []
    for e in range(NE):
        sel1 = np.nonzero(top2_t[:, 0] == e)[0]
        sel2 = np.nonzero(top2_t[:, 1] == e)[0]
        toks = np.concatenate([sel1, sel2])
        ws = np.concatenate([w_t[sel1, 0], w_t[sel2, 1]])
        o = np.argsort(toks, kind="stable")
        glist.append(toks[o])
        gw.append(ws[o].astype(np.float32))
    counts = [len(g) for g in glist]

    C, sizes, parts = _plan_xp(counts)
    nslots = len(sizes)
    if ("xp", C, tuple(sizes)) not in _cache:
        _cache[("xp", C, tuple(sizes))] = _build_xp(C, sizes)
    nc = _cache[("xp", C, tuple(sizes))]

    # assign chunks to (core, slot): class i chunk list padded to 8 with
    # dummies; big primary chunks paired with small secondary chunks.
    used = [0] * NE  # groups of expert e already assigned
    asn = [[None] * nslots for _ in range(NCORES)]
    for i in range(nslots):
        chunk_list = list(parts[i]) + [(0, 0)] * (NCORES - len(parts[i]))
        if i == 0:
            chunk_list.sort(key=lambda t: -t[1])
        else:
            chunk_list.sort(key=lambda t: t[1])
        for c in range(NCORES):
            asn[c][i] = chunk_list[c]
    # materialize token ranges in class-major deterministic order
    core_parts = [[] for _ in range(NCORES)]  # (slot, expert, tok_start, n_tok)
    for i in range(nslots):
        for c in range(NCORES):
            e, ngr = asn[c][i]
            tok0 = used[e] * GRAN if ngr else 0
            ntok = min(counts[e] - tok0, ngr * GRAN) if ngr else 0
            ntok = max(ntok, 0)
            if ngr:
                used[e] += ngr
            core_parts[c].append((i, e, tok0, ntok))

    we_h = [
        np.ascontiguousarray(
            We[e].reshape(4, 128, 4, 128).transpose(1, 2, 0, 3)
        ).reshape(128, 16 * 128).astype(np.float16)
        for e in range(NE)
    ]  # [c, b, k, d]
    bias_h = [np.ascontiguousarray(be[e].reshape(4, 128).T) for e in range(NE)]

    in_maps = []
    for c in range(NCORES):
        tok_ids = np.zeros(C * GRAN, np.int64)
        valid = np.zeros(C * GRAN, bool)
        g_base = 0
        we_np = np.zeros((128, nslots * 16 * 128), np.float16)
        bias_np = np.zeros((128, nslots * 4), np.float32)
        for (i, e, tok0, ntok) in core_parts[c]:
            sl0 = g_base * GRAN
            tok_ids[sl0 : sl0 + ntok] = glist[e][tok0 : tok0 + ntok]
            valid[sl0 : sl0 + ntok] = True
            we_np[:, i * 16 * 128 : (i + 1) * 16 * 128] = we_h[e]
            bias_np[:, i * 4 : (i + 1) * 4] = bias_h[e]
            g_base += sizes[i]
        xg = x2[tok_ids].astype(np.float16)
        xg[~valid] = 0
        # [slot(C*GRAN), din] -> [c, g, k, t]
        xg_np = np.ascontiguousarray(
            xg.reshape(C, GRAN, 4, 128).transpose(3, 0, 2, 1)
        ).reshape(128, C * 4 * GRAN)
        in_maps.append(
            {"xg": xg_np, "we": we_np, "bias": bias_np}
        )

    res = run_bass_kernel_spmd(nc, in_maps, list(range(NCORES)))
    last_results = res

    out_full = np.zeros((T, DOUT), np.float32)
    for c in range(NCORES):
        yg = np.asarray(res.results[c]["yg"])
        # [128(d_low), C, 4(b), GRAN] -> rows [C*GRAN, 512]
        rows = (
            yg.reshape(128, C, 4, GRAN)
            .transpose(1, 3, 2, 0)
            .reshape(C * GRAN, DOUT)
            .astype(np.float32)
        )
        g_base = 0
        for (i, e, tok0, ntok) in core_parts[c]:
            sl0 = g_base * GRAN
            if ntok:
                ids = glist[e][tok0 : tok0 + ntok]
                ws = gw[e][tok0 : tok0 + ntok]
                out_full[ids] += ws[:, None] * rows[sl0 : sl0 + ntok]
            g_base += sizes[i]
    return out_full.reshape(NB, NLOC, DOUT)


def kernel(x, type_embeddings, atom_types, Wg, We, be):
    global last_results
    design = os.environ.get("MOE_DESIGN", "xp")
    if design == "xp":
        try:
            return _kernel_xp(x, type_embeddings, atom_types, Wg, We, be)
        except Exception:
            # planner/build failure on unusual routing distributions: fall
            # back to the slower but shape-robust routed design
            return _kernel_routed(x, type_embeddings, atom_types, Wg, We, be)
    if design == "routed2":
        return _kernel_routed2(x, type_embeddings, atom_types, Wg, We, be)
    if design == "routed":
        return _kernel_routed(x, type_embeddings, atom_types, Wg, We, be)
    x = np.asarray(x, np.float32)
    We = np.asarray(We, np.float32)
    be = np.asarray(be, np.float32)
    ptw, _, _ = _routing(
        np.asarray(type_embeddings, np.float32),
        np.asarray(Wg, np.float32),
        np.asarray(atom_types),
    )

    x2 = x.reshape(T, DIN)
    ber = np.ascontiguousarray(
        np.broadcast_to(be.reshape(1, NE * DOUT), (128, NE * DOUT))
    )
    # [128, NE*KT*DOUT]: we_h[p, (e*KT+k)*DOUT + d] = We[e, k*128+p, d]
    we_h = np.ascontiguousarray(
        We.reshape(NE, KT, 128, DOUT).transpose(2, 0, 1, 3)
    ).reshape(128, NE * KT * DOUT)
    in_maps = []
    for c in range(NCORES):
        x2c = x2[c * TC : (c + 1) * TC]
        # [128, KT*TC]: xt[p, k*TC + n] = x2c[n, k*128+p]
        xt = np.ascontiguousarray(
            x2c.reshape(TC, KT, 128).transpose(2, 1, 0)
        ).reshape(128, KT * TC)
        xw = np.concatenate([xt, we_h], axis=1)
        pwl = np.ascontiguousarray(
            ptw[c * TC : (c + 1) * TC].reshape(MT, 128, NE).transpose(1, 0, 2)
        ).reshape(128, MT * NE)
        in_maps.append({"xw": xw, "pwl": pwl, "ber": ber})

    if "dense" not in _cache:
        _cache["dense"] = _build_dense()
    nc = _cache["dense"]

    res = run_bass_kernel_spmd(nc, in_maps, list(range(NCORES)))
    last_results = res
    out = np.concatenate([res.results[c]["out"] for c in range(NCORES)], axis=0)
    return out.reshape(NB, NLOC, DOUT).astype(np.float32)

